# revision 1
# baseline (speedup 1.0000x reference)
"""Trainium2 Bass kernel for the 2-layer LSTM encoder/decoder problem.

Strategy (8 NeuronCores):
  - Tensor-parallel shard of the 4L=8192 gate rows: core k owns rows
    [256k:256k+256) of each gate (i,f,g,o) -> 1024 gate rows / core.
  - Activations live transposed [feature, batch] on device; batch = 32
    (the two independent scan chains of the reference are batched).
  - Non-autoregressive phases (encoder scans, decoder consume scans) are
    processed layer-by-layer: the Wih contribution for all 4 timesteps is
    one M=128 bulk matmul (weights stream once); only the Whh recurrence
    is stepwise, with Whh SBUF-resident.
  - Matmuls run in bf16 (PSUM accumulation fp32); cell state and outputs
    stay fp32. All four decoder matrices are SBUF-resident in bf16, so
    the autoregressive phase does no weight streaming at all.
  - Hidden slices are AllGather'ed (bf16) between layer-steps; chunk
    outputs are written per-core (own slice, fp32) and gathered on host.
  - Features are globally permuted f' = 512*h + c so the final 1x1 conv
    is a plain matmul over the gathered hidden tiles.
"""

import tempfile

import numpy as np
import ml_dtypes

import concourse.bass as bass
import concourse.bacc as bacc
import concourse.mybir as mybir
import concourse.tile as tile
from concourse import bass_utils

# Problem constants (hardcoded per contract)
C, H, W = 512, 4, 4
SPLIT, PRED = 4, 4
L = 2048           # lstm feature size
B = 16             # reference batch
NB = 32            # device batch (two chains)
NCORES = 8
SL = L // NCORES   # 256: hidden slice per core
GL = 4 * SL        # 1024: gate rows per core
NT = L // 128      # 16 k-tiles
NAR = PRED + SPLIT - 1  # 7 autoregressive steps

F32 = mybir.dt.float32
BF16 = mybir.dt.bfloat16
NPBF = ml_dtypes.bfloat16

# Permutation: device feature f' = 512*h + c  <->  natural f = 4*c + h
PERM = np.array([4 * (f % C) + f // C for f in range(L)], dtype=np.int64)
IPERM = np.argsort(PERM)

_CACHE = {}


def _build_nc():
    nc = bacc.Bacc("TRN2", target_bir_lowering=False, debug=False,
                   num_devices=NCORES)

    def din(name, shape, dt=F32):
        return nc.dram_tensor(name, shape, dt, kind="ExternalInput").ap()

    def dout(name, shape):
        return nc.dram_tensor(name, shape, F32, kind="ExternalOutput").ap()

    xET = din("xET", [128, NT * 4 * NB], BF16)
    xDT = din("xDT", [128, NT * 4 * NB], BF16)
    eWih = din("eWih", [2, L, GL], BF16)
    eWhh = din("eWhh", [2, L, GL], BF16)
    dWih = din("dWih", [2, L, GL], BF16)
    dWhh = din("dWhh", [2, L, GL], BF16)
    eB = din("eB", [2, 32, GL])       # bias replicated over 32 partitions
    dB = din("dB", [2, 32, GL])
    cWT = din("cWT", [2 * C, C], BF16)      # conv_W.T
    cB = din("cB", [64, C])           # conv bias replicated over 64 rows

    # per-core own h2 slice [32 batch, 256 feat] per chunk; host gathers
    chunks_out = dout("chunks_out", [8, NB, SL])
    convout = dout("convout", [4, 64, C])   # [w, (h,b), out_ch]

    with tile.TileContext(nc) as tc:
        with (
            tc.tile_pool(name="bias", bufs=3) as biasp,
            tc.tile_pool(name="whh", bufs=4) as whhp,
            tc.tile_pool(name="cwt", bufs=1) as cwtp,
            tc.tile_pool(name="wstr", bufs=3) as wstrp,
            tc.tile_pool(name="x2t", bufs=1) as x2tp,
            tc.tile_pool(name="xin", bufs=1) as xinp,
            tc.tile_pool(name="usb", bufs=1) as usbp,
            tc.tile_pool(name="ut", bufs=3) as utp,
            tc.tile_pool(name="h2big", bufs=7) as h2bigp,
            tc.tile_pool(name="h1big", bufs=2) as h1bigp,
            tc.tile_pool(name="gw", bufs=1) as gwp,
            tc.tile_pool(name="cst", bufs=2) as cstp,
            tc.tile_pool(name="hsl", bufs=1) as hslp,
            tc.tile_pool(name="psu", bufs=2, space="PSUM") as psup,
            tc.tile_pool(name="psg", bufs=4, space="PSUM") as psgp,
            tc.tile_pool(name="dram", bufs=3, space="DRAM") as dramp,
        ):
            def load_bias(src, l, name):
                t_ = biasp.tile([32, GL], F32, tag="bias", name=name)
                nc.sync.dma_start(t_[:], src[l])
                return t_

            def load_w(w_dram, l, name, eng=None):
                # resident weight matrix -> [128, NT*GL] layout [p, kt*GL+n]
                wt = whhp.tile([128, NT * GL], BF16, tag="whh", name=name)
                (eng or nc.scalar).dma_start(
                    wt[:].rearrange("p (kt n) -> p kt n", kt=NT),
                    w_dram[l].rearrange("(kt p) n -> p kt n", p=128),
                )
                return wt

            def ag(ht):
                """AllGather this core's [256, 32] h slice -> [2048, 32].
                ht: [32, SL] block-transposed (block q col b row j =
                h[b, 32q+j]); cin[32q+j, b] = ht[j, 32q+b]."""
                cin = dramp.tile([2 * 128, NB], BF16, tag="agin")
                nc.sync.dma_start(
                    cin.rearrange("(q j) b -> j q b", j=32),
                    ht[:].rearrange("j (q b) -> j q b", b=NB))
                cout = dramp.tile([L, NB], BF16, tag="agout",
                                  addr_space="Shared")
                nc.gpsimd.collective_compute(
                    "AllGather", mybir.AluOpType.bypass,
                    replica_groups=[list(range(NCORES))],
                    ins=[cin[:]], outs=[cout[:]],
                )
                return cout

            def big_from_ag(cout, pool, tag):
                # SBUF bigtile [128, NT*32] layout [p, kt*32 + b]
                bt = pool.tile([128, NT * NB], BF16, tag=tag)
                nc.sync.dma_start(
                    bt[:].rearrange("p (kt b) -> p kt b", kt=NT),
                    cout.rearrange("(kt p) b -> p kt b", p=128),
                )
                return bt

            def x2t_from_ag(cout, x2t, t):
                # write h1T of step t into X2T columns kt*128 + t*32
                nc.sync.dma_start(
                    x2t[:].rearrange("p (kt t b) -> p kt t b",
                                     kt=NT, t=4)[:, :, t, :],
                    cout.rearrange("(kt p) b -> p kt b", p=128),
                )

            SIG = mybir.ActivationFunctionType.Sigmoid
            TANH = mybir.ActivationFunctionType.Tanh

            def cell(gsrc, add_ap, c_old, ltag, add2_ap=None, out_idx=None):
                """LSTM cell elementwise, partition base 0.
                gsrc: [psg0, psg1] PSUM pair or single [32, GL] AP.
                Returns (c_new, ht) with ht the bf16 block-transposed
                [32, SL] h slice ready for AllGather."""
                if isinstance(gsrc, (list, tuple)):
                    halves = [gsrc[0][:], gsrc[1][:]]
                else:
                    halves = [gsrc[:, 0:512], gsrc[:, 512:GL]]
                if add_ap is not None:
                    # separate half-tiles so ACT on half0 doesn't wait the
                    # DVE add of half1 (Tile deps are per-tile)
                    ga = gwp.tile([32, 512], F32, tag="ga")
                    gb = gwp.tile([32, 512], F32, tag="gb")
                    nc.vector.tensor_add(ga[:], halves[0], add_ap[:, 0:512])
                    nc.vector.tensor_add(gb[:], halves[1], add_ap[:, 512:GL])
                    if add2_ap is not None:
                        nc.vector.tensor_add(ga[:], ga[:], add2_ap[:, 0:512])
                        nc.vector.tensor_add(gb[:], gb[:], add2_ap[:, 512:GL])
                    halves = [ga[:], gb[:]]
                act = nc.scalar.activation
                if_t = gwp.tile([32, 2 * SL], F32, tag="ift")
                gt_t = gwp.tile([32, SL], F32, tag="gtt")
                o_t = gwp.tile([32, SL], F32, tag="ot")
                i_s = if_t[:, 0:SL]
                f_s = if_t[:, SL:2 * SL]
                act(if_t[:], halves[0], SIG)           # i, f fused
                act(gt_t[:], halves[1][:, 0:SL], TANH)
                act(o_t[:], halves[1][:, SL:2 * SL], SIG)
                c_new = cstp.tile([32, SL], F32, tag="c" + ltag)
                tmp = gwp.tile([32, SL], F32, tag="tmp")
                nc.vector.tensor_mul(tmp[:], i_s, gt_t[:])
                if c_old is not None:
                    cmul = gwp.tile([32, SL], F32, tag="cmul")
                    nc.vector.tensor_mul(cmul[:], f_s, c_old[:])
                    nc.vector.tensor_add(c_new[:], cmul[:], tmp[:])
                else:
                    nc.vector.tensor_copy(c_new[:], tmp[:])
                tanh_c = gwp.tile([32, SL], F32, tag="tanhc")
                act(tanh_c[:], c_new[:], TANH)
                hb = gwp.tile([32, SL], BF16, tag="hb")
                if out_idx is not None:
                    nc.vector.tensor_mul(tmp[:], o_t[:], tanh_c[:])
                    nc.sync.dma_start(chunks_out[out_idx], tmp[:])
                    nc.vector.tensor_copy(hb[:], tmp[:])
                else:
                    nc.vector.tensor_mul(hb[:], o_t[:], tanh_c[:])
                ht = hslp.tile([32, SL], BF16, tag="hsl")
                nc.vector.transpose(ht[:], hb[:])
                return c_new, ht

            def bulk_u(lhs_fn, w_dram, l, bias_ap):
                """U[t] = X[t] @ Wih_l^T for 4 steps; uts[1..3] are base-0
                [32, GL] fp32 tiles incl. bias; uts[0] None (u_sb[0:32])."""
                psums = [psup.tile([128, 512], F32, tag="psu",
                                   name=f"psu{n_}") for n_ in range(2)]
                for kt in range(NT):
                    wt = wstrp.tile([128, GL], BF16, tag="wstr")
                    nc.scalar.dma_start(
                        wt[:], w_dram[l, kt * 128:(kt + 1) * 128, :])
                    lhs = lhs_fn(kt)
                    for n in range(2):
                        nc.tensor.matmul(
                            psums[n][:], lhs, wt[:, n * 512:(n + 1) * 512],
                            start=(kt == 0), stop=(kt == NT - 1),
                            skip_group_check=True)
                u_sb = usbp.tile([128, GL], F32, tag="usb")
                for n in range(2):
                    nc.vector.tensor_copy(u_sb[:, n * 512:(n + 1) * 512],
                                          psums[n][:])
                uts = [None] * 4
                for t in range(1, 4):
                    ut = utp.tile([32, GL], F32, tag="ut")
                    nc.sync.dma_start(ut[:], u_sb[32 * t:32 * t + 32, :])
                    nc.vector.tensor_add(ut[:], ut[:], bias_ap)
                    uts[t] = ut
                return u_sb, uts

            def whh_matmuls(h_lhs_fn, whh_sb, extra=None):
                """K=2048 accumulation vs resident weights -> [psg0, psg1].
                extra: (lhs_fn, wih_sb) second K=2048 accumulation."""
                psums = [psgp.tile([32, 512], F32, tag="psg",
                                   name=f"psg{n_}") for n_ in range(2)]
                for kt in range(NT):
                    lhs = h_lhs_fn(kt)
                    for n in range(2):
                        nc.tensor.matmul(
                            psums[n][:], lhs,
                            whh_sb[:, kt * GL + n * 512:
                                   kt * GL + n * 512 + 512],
                            start=(kt == 0),
                            stop=(extra is None and kt == NT - 1),
                            skip_group_check=True)
                if extra is not None:
                    lhs2, wih_sb = extra
                    for kt in range(NT):
                        lhs = lhs2(kt)
                        for n in range(2):
                            nc.tensor.matmul(
                                psums[n][:], lhs,
                                wih_sb[:, kt * GL + n * 512:
                                       kt * GL + n * 512 + 512],
                                start=False, stop=(kt == NT - 1),
                                skip_group_check=True)
                return psums

            def load_xin(x_dram, name):
                # host pre-laid-out [p, kt*128 + t*32 + b]; one linear DMA
                t_ = xinp.tile([128, NT * 128], BF16, tag="xin", name=name)
                nc.sync.dma_start(t_[:], x_dram[:, :])
                return t_

            def x2t_block(x2t, kt, t):
                return x2t[:, kt * 128 + 32 * t: kt * 128 + 32 * t + 32]

            def wih_matmuls(lhs_fn, wih_sb):
                """K=2048 accumulation vs resident Wih only (layer-1 t0)."""
                psums = [psgp.tile([32, 512], F32, tag="psg",
                                   name=f"psgw{n_}") for n_ in range(2)]
                for kt in range(NT):
                    lhs = lhs_fn(kt)
                    for n in range(2):
                        nc.tensor.matmul(
                            psums[n][:], lhs,
                            wih_sb[:, kt * GL + n * 512:
                                   kt * GL + n * 512 + 512],
                            start=(kt == 0), stop=(kt == NT - 1),
                            skip_group_check=True)
                return psums

            def dual_scan(l0_init_lhs, uts, u_sb, whh0, b0, whh1, wih1, b1,
                          c1_init, c2_init, x2t_out, h2_init_big,
                          zero_init, store_de):
                """Wavefront over both layers: layer-0 (bulk-U + Whh0) and
                layer-1 (step-wise: Wih1 @ h1_t + Whh1 @ h2_{t-1}).
                Layer-1's matmuls fill layer-0's AllGather windows."""
                c1p, c2p = c1_init, c2_init
                h2_prev = h2_init_big
                for t in range(4):
                    # ---- layer 0 step t ----
                    if zero_init and t == 0:
                        c1p, ht = cell(u_sb[0:32, :], b0, None, "1")
                    else:
                        if t == 0:
                            lhs = l0_init_lhs
                        else:
                            lhs = lambda kt: x2t_block(x2t_out, kt, t - 1)
                        psums = whh_matmuls(lhs, whh0)
                        if uts[t] is not None:
                            c1p, ht = cell(psums, uts[t][:], c1p, "1")
                        else:
                            c1p, ht = cell(psums, u_sb[0:32, :], c1p, "1",
                                           add2_ap=b0)
                    cout = ag(ht)
                    x2t_from_ag(cout, x2t_out, t)
                    # ---- layer 1 step t ----
                    wih_lhs = lambda kt: x2t_block(x2t_out, kt, t)
                    if zero_init and t == 0:
                        psums = wih_matmuls(wih_lhs, wih1)
                        c2p, ht = cell(psums, b1, None, "2")
                    else:
                        h2b = h2_prev
                        psums = whh_matmuls(
                            lambda kt: h2b[:, kt * NB:kt * NB + NB], whh1,
                            extra=(wih_lhs, wih1))
                        c2p, ht = cell(
                            psums, b1, c2p, "2",
                            out_idx=(0 if store_de and t == 3 else None))
                    cout = ag(ht)
                    h2_prev = big_from_ag(cout, h2bigp, "h2big")
                return c1p, c2p, h2_prev

            # =========================================================
            # Phase E: encoder (batch 32 = [x2 fwd chain, x1-rev chain])
            # =========================================================
            eb0 = load_bias(eB, 0, "eb0")
            eb1 = load_bias(eB, 1, "eb1")
            whh_e0 = load_w(eWhh, 0, "whh_e0", eng=nc.sync)
            xe_sb = load_xin(xET, "xe_sb")
            u_sb, uts = bulk_u(
                lambda kt: xe_sb[:, kt * 128:(kt + 1) * 128], eWih, 0, eb0[:])
            whh_e1 = load_w(eWhh, 1, "whh_e1")
            wih_e1 = load_w(eWih, 1, "wih_e1")
            x2t_e = x2tp.tile([128, NT * 128], BF16, tag="x2t")
            c_e1, c_e2, h2_big = dual_scan(
                None, uts, u_sb, whh_e0, eb0[:], whh_e1, wih_e1, eb1[:],
                None, None, x2t_e, None, True, False)

            # =========================================================
            # Phase D1: decoder consume (batch = [x1 fwd, x2-rev])
            # =========================================================
            db0 = load_bias(dB, 0, "db0")
            db1 = load_bias(dB, 1, "db1")
            whh_d0 = load_w(dWhh, 0, "whh_d0")
            xd_sb = load_xin(xDT, "xd_sb")
            u_sb, uts = bulk_u(
                lambda kt: xd_sb[:, kt * 128:(kt + 1) * 128], dWih, 0, db0[:])
            whh_d1 = load_w(dWhh, 1, "whh_d1")
            wih_d1 = load_w(dWih, 1, "wih_d1")
            wih_d0 = load_w(dWih, 0, "wih_d0")
            x2t_d = x2tp.tile([128, NT * 128], BF16, tag="x2t")
            c1, c2, h2_big = dual_scan(
                lambda kt: x2t_block(x2t_e, kt, 3), uts, u_sb,
                whh_d0, db0[:], whh_d1, wih_d1, db1[:],
                c_e1, c_e2, x2t_d, h2_big, False, True)

            def emit_conv(cwt_sb, cb_sb):
                b1 = [conv_tiles["de"], conv_tiles["ar0"],
                      conv_tiles["ar1"], conv_tiles["ar2"]]
                b2 = [conv_tiles["ar2"], conv_tiles["ar1"],
                      conv_tiles["ar0"], conv_tiles["de"]]
                for w in range(4):
                    pcv = psup.tile([128, 512], F32, tag="psu",
                                    name=f"pcv{w}")
                    first = True
                    for br, src in ((0, b1[w]), (1, b2[w])):
                        lhs = src[:].rearrange("p (kt b) -> p kt b", kt=NT)
                        for j in range(4):
                            st = gwp.tile([128, 64], BF16, tag="cvl", bufs=2,
                                          name=f"cvl{w}_{br}_{j}")
                            nc.vector.tensor_copy(
                                st[:].rearrange("p (h b) -> p h b", h=4),
                                lhs[:, j::4, 16 * br:16 * br + 16])
                            nc.tensor.matmul(
                                pcv[0:64, :], st[:],
                                cwt_sb[:, (4 * br + j) * C:
                                       (4 * br + j + 1) * C],
                                start=first, stop=(br == 1 and j == 3))
                            first = False
                    cvs = gwp.tile([64, C], F32, tag="g", name=f"cvs{w}")
                    nc.vector.tensor_add(cvs[:], pcv[0:64, :], cb_sb[:])
                    cvo = gwp.tile([64, C], F32, tag="g2", name=f"cvo{w}")
                    nc.vector.tensor_scalar_mul(cvo[:], cvs[:], 0.2)
                    nc.vector.tensor_max(cvo[:], cvo[:], cvs[:])
                    nc.sync.dma_start(convout[w], cvo[:])

            # =========================================================
            # Phase D2: autoregressive decoder (7 steps, zero streaming)
            # =========================================================
            conv_tiles = {"de": h2_big}
            h1_big = None
            cb_sb = biasp.tile([64, C], F32, tag="bias", name="cb_sb")
            nc.sync.dma_start(cb_sb[:], cB[:])
            cwt_sb = cwtp.tile([128, 8 * C], BF16, tag="cwt")
            nc.scalar.dma_start(
                cwt_sb[:].rearrange("p (j o) -> p j o", j=8),
                cWT.rearrange("(j p) o -> p j o", p=128))

            for t in range(NAR):
                if t == 3:
                    emit_conv(cwt_sb, cb_sb)
                h2b, h1b = h2_big, h1_big
                if t == 0:
                    l0_lhs = lambda kt: x2t_block(x2t_d, kt, 3)
                else:
                    l0_lhs = lambda kt: h1b[:, kt * NB:kt * NB + NB]
                psums = whh_matmuls(
                    l0_lhs, whh_d0,
                    extra=(lambda kt: h2b[:, kt * NB:kt * NB + NB], wih_d0))
                c1, ht = cell(psums, db0[:], c1, "1")
                cout = ag(ht)
                h1_big = big_from_ag(cout, h1bigp, "h1big")

                h1b2 = h1_big
                psums = whh_matmuls(
                    lambda kt: h2b[:, kt * NB:kt * NB + NB], whh_d1,
                    extra=(lambda kt: h1b2[:, kt * NB:kt * NB + NB], wih_d1))
                c2, ht = cell(psums, db1[:], c2, "2", out_idx=t + 1)
                if t < NAR - 1:
                    cout = ag(ht)
                    h2_big = big_from_ag(cout, h2bigp, "h2big")
                    if t < 3:
                        conv_tiles[f"ar{t}"] = h2_big

    nc.compile()
    return nc


def _prep_inputs(x1, x2, enc_Wih, enc_Whh, enc_bih, enc_bhh,
                 dec_Wih, dec_Whh, dec_bih, dec_bhh, conv_W, conv_b):
    def colvecs(x):
        return [np.ascontiguousarray(x[:, :, :, t].reshape(B, L))
                for t in range(4)]

    x1c, x2c = colvecs(x1), colvecs(x2)

    def ximg(xa):
        # [4, L, NB] -> SBUF image [128, kt*128 + t*32 + b]
        return np.ascontiguousarray(
            xa.reshape(4, NT, 128, NB).transpose(2, 1, 0, 3)
            .reshape(128, NT * 4 * NB)).astype(NPBF)

    xET = ximg(np.stack([
        np.concatenate([x2c[t], x1c[3 - t]], axis=0)[:, PERM].T
        for t in range(4)]))
    xDT = ximg(np.stack([
        np.concatenate([x1c[t], x2c[3 - t]], axis=0)[:, PERM].T
        for t in range(4)]))

    def prep_core(k, Wih, Whh, bih, bhh):
        rows = np.concatenate([g * L + PERM[k * SL:(k + 1) * SL]
                               for g in range(4)])
        wihT = np.stack([np.ascontiguousarray(Wih[l][rows][:, PERM].T)
                         for l in range(2)])
        whhT = np.stack([np.ascontiguousarray(Whh[l][rows][:, PERM].T)
                         for l in range(2)])
        bb = np.stack([(bih[l] + bhh[l])[rows] for l in range(2)])
        brep = np.broadcast_to(bb[:, None, :], (2, 32, GL)).copy()
        return wihT.astype(NPBF), whhT.astype(NPBF), brep.astype(np.float32)

    cWT = np.ascontiguousarray(conv_W.T).astype(NPBF)
    cBr = np.broadcast_to(conv_b[None, :], (64, C)).copy().astype(np.float32)

    in_maps = []
    for k in range(NCORES):
        eWihT, eWhhT, eBr = prep_core(k, enc_Wih, enc_Whh, enc_bih, enc_bhh)
        dWihT, dWhhT, dBr = prep_core(k, dec_Wih, dec_Whh, dec_bih, dec_bhh)
        in_maps.append({
            "xET": xET, "xDT": xDT,
            "eWih": eWihT, "eWhh": eWhhT, "eB": eBr,
            "dWih": dWihT, "dWhh": dWhhT, "dB": dBr,
            "cWT": cWT, "cB": cBr,
        })
    return in_maps


def _postprocess(results, x1, x2):
    # gather chunk slices across cores: core k owns features [256k:256k+256)
    chunks = np.zeros((8, B * 2, L), np.float32)
    for k in range(NCORES):
        chunks[:, :, k * SL:(k + 1) * SL] = results[k]["chunks_out"]
    convout = results[0]["convout"]

    def tochunk(t, half):
        v = chunks[t, half * B:(half + 1) * B, :]   # [16, L] dev order
        return v[:, IPERM].reshape(B, C, H)

    de1 = tochunk(0, 0)
    p1 = [tochunk(1 + j, 0) for j in range(NAR)]
    de2 = tochunk(0, 1)
    p2 = [tochunk(1 + j, 1) for j in range(NAR)]

    mid1 = np.stack([de1, p1[0], p1[1], p1[2]], axis=-1)
    tail1 = np.stack([p1[3], p1[4], p1[5], p1[6]], axis=-1)
    head2 = np.stack([p2[6], p2[5], p2[4], p2[3]], axis=-1)
    mid2 = np.stack([p2[2], p2[1], p2[0], de2], axis=-1)

    out = convout.reshape(4, 4, B, C).transpose(2, 3, 1, 0)
    out = np.ascontiguousarray(out, dtype=np.float32)
    return (out, np.asarray(x1), mid1, tail1, head2, mid2, np.asarray(x2))


def _run(in_maps, trace=False):
    if "nc" not in _CACHE:
        _CACHE["nc"] = _build_nc()
        _CACHE["tmpdir"] = tempfile.mkdtemp(prefix="lstmk_")
    nc = _CACHE["nc"]
    res = bass_utils.run_bass_kernel_spmd(
        nc, in_maps, core_ids=list(range(NCORES)), trace=trace,
        tmpdir=_CACHE["tmpdir"] if trace else None)
    return res


def kernel(**inputs):
    inputs = {k: np.asarray(v, dtype=np.float32) for k, v in inputs.items()}
    in_maps = _prep_inputs(**inputs)
    res = _run(in_maps, trace=False)
    return _postprocess(res.results, inputs["x1"], inputs["x2"])


def kernel_traced(**inputs):
    inputs = {k: np.asarray(v, dtype=np.float32) for k, v in inputs.items()}
    in_maps = _prep_inputs(**inputs)
    res = _run(in_maps, trace=True)
    return _postprocess(res.results, inputs["x1"], inputs["x2"]), res



# revision 9
# speedup vs baseline: 1.1011x; 1.1011x over previous
"""Trainium2 Bass kernel for the 2-layer LSTM encoder/decoder problem.

Strategy (8 NeuronCores):
  - Tensor-parallel shard of the 4L=8192 gate rows: core k owns rows
    [256k:256k+256) of each gate (i,f,g,o) -> 1024 gate rows / core.
  - Activations live transposed [feature, batch] on device; batch = 32
    (the two independent scan chains of the reference are batched).
  - Non-autoregressive phases are processed layer-by-layer: the Wih
    contribution for all 4 timesteps is one M=128 bulk matmul; only the
    Whh recurrence is stepwise, with Whh SBUF-resident.
  - Matmuls run in bf16 (PSUM accumulation fp32); cell state stays fp32.
  - Gate biases enter PSUM via K=1 matmuls (ones-vector x bias row), so
    the LSTM cell activations read PSUM directly with no DVE adds.
  - Hidden slices are AllGather'ed (bf16) between layer-steps in a
    [128 x 64] per-rank layout (128B DMA lines); the gathered tensor is
    read back as two half-tiles on separate DMA queues so dependent
    matmuls start after half the transfer.
  - A dummy warmup AllGather at kernel start absorbs the cold ncfw
    barrier into the initial weight-load window.
  - The 1x1-conv epilogue is interleaved into the autoregressive loop's
    tensor-engine idle windows (one output column per iteration).
"""

import tempfile

import numpy as np
import ml_dtypes

import concourse.bass as bass
import concourse.bacc as bacc
import concourse.mybir as mybir
import concourse.tile as tile
from concourse import bass_utils

# Problem constants (hardcoded per contract)
C, H, W = 512, 4, 4
SPLIT, PRED = 4, 4
L = 2048           # lstm feature size
B = 16             # reference batch
NB = 32            # device batch (two chains)
NCORES = 8
SL = L // NCORES   # 256: hidden slice per core
GL = 4 * SL        # 1024: gate rows per core
NT = L // 128      # 16 k-tiles
NAR = PRED + SPLIT - 1  # 7 autoregressive steps

F32 = mybir.dt.float32
BF16 = mybir.dt.bfloat16
NPBF = ml_dtypes.bfloat16

# Permutation: device feature f' = 512*h + c  <->  natural f = 4*c + h
PERM = np.array([4 * (f % C) + f // C for f in range(L)], dtype=np.int64)
IPERM = np.argsort(PERM)

_CACHE = {}


def _build_nc():
    nc = bacc.Bacc("TRN2", target_bir_lowering=False, debug=False,
                   num_devices=NCORES)

    def din(name, shape, dt=F32):
        return nc.dram_tensor(name, shape, dt, kind="ExternalInput").ap()

    def dout(name, shape, dt=F32):
        return nc.dram_tensor(name, shape, dt, kind="ExternalOutput").ap()

    xET = din("xET", [128, NT * 4 * NB], BF16)
    xDT = din("xDT", [128, NT * 4 * NB], BF16)
    eWih = din("eWih", [2, L, GL], BF16)
    eWhh = din("eWhh", [2, L, GL], BF16)
    dWih = din("dWih", [2, L, GL], BF16)
    dWhh = din("dWhh", [2, L, GL], BF16)
    eB = din("eB", [2, 1, GL], BF16)      # bias row (enters psum via matmul)
    dB = din("dB", [2, 1, GL], BF16)
    cWT = din("cWT", [2 * C, C], BF16)    # conv_W.T
    cB = din("cB", [64, C])               # conv bias replicated over 64 rows

    # per-core own h2 slice [32 batch, 256 feat] per chunk; host gathers
    chunks_out = dout("chunks_out", [8, NB, SL], BF16)
    convout = dout("convout", [4, 64, C])   # [w, (h,b), out_ch]

    with tile.TileContext(nc) as tc:
        with (
            tc.tile_pool(name="bias", bufs=4) as biasp,
            tc.tile_pool(name="whh", bufs=4) as whhp,
            tc.tile_pool(name="cwt", bufs=1) as cwtp,
            tc.tile_pool(name="wstr", bufs=3) as wstrp,
            tc.tile_pool(name="x2t", bufs=1) as x2tp,
            tc.tile_pool(name="xin", bufs=1) as xinp,
            tc.tile_pool(name="usb", bufs=1) as usbp,
            tc.tile_pool(name="ut", bufs=3) as utp,
            tc.tile_pool(name="h2lo", bufs=7) as h2lop,
            tc.tile_pool(name="h2hi", bufs=7) as h2hip,
            tc.tile_pool(name="h1lo", bufs=2) as h1lop,
            tc.tile_pool(name="h1hi", bufs=2) as h1hip,
            tc.tile_pool(name="gw", bufs=1) as gwp,
            tc.tile_pool(name="cst", bufs=2) as cstp,
            tc.tile_pool(name="hsl", bufs=2) as hslp,
            tc.tile_pool(name="psu", bufs=2, space="PSUM") as psup,
            tc.tile_pool(name="psg", bufs=4, space="PSUM") as psgp,
            tc.tile_pool(name="dram", bufs=3, space="DRAM") as dramp,
        ):
            RG = [list(range(NCORES))]

            # ---- warmup collective: absorb cold barrier into startup ----
            wsb = gwp.tile([1, 64], BF16, tag="warm")
            nc.gpsimd.memset(wsb[:], 0.0)
            win = dramp.tile([1, 64], BF16, tag="warmin")
            nc.gpsimd.dma_start(win[:], wsb[:])
            wout = dramp.tile([NCORES, 64], BF16, tag="warmout",
                              addr_space="Shared")
            nc.gpsimd.collective_compute(
                "AllGather", mybir.AluOpType.bypass,
                replica_groups=RG, ins=[win[:]], outs=[wout[:]])

            # ones row for bias-into-psum matmuls
            ones_sb = biasp.tile([1, 128], BF16, tag="ones", bufs=1)
            nc.gpsimd.memset(ones_sb[:], 1.0)

            def load_bias(src, l, name):
                t_ = biasp.tile([1, GL], BF16, tag="bias", name=name)
                nc.gpsimd.dma_start(t_[:], src[l])
                return t_

            def load_w(w_dram, l, name, eng):
                # resident weight matrix -> [128, NT*GL] layout [p, kt*GL+n]
                wt = whhp.tile([128, NT * GL], BF16, tag="whh", name=name)
                eng.dma_start(
                    wt[:].rearrange("p (kt n) -> p kt n", kt=NT),
                    w_dram[l].rearrange("(kt p) n -> p kt n", p=128),
                )
                return wt

            def ag(ht):
                """AllGather this core's h slice.  ht: [32, SL] block-
                transposed (ht[j, 32q+b] = h[b, 32q+j]).  cin is laid out
                [128, 64]: cin[32*q4+j, 32*k2+b] = h[b, 128*k2+32*q4+j],
                so gathered rank blocks give 128B-contiguous lines per
                SBUF partition on readback."""
                cin = dramp.tile([128, 2 * NB], BF16, tag="agin")
                nc.gpsimd.dma_start(
                    cin.rearrange("(q4 j) (k2 b) -> j k2 q4 b", j=32, k2=2),
                    ht[:].rearrange("j (k2 q4 b) -> j k2 q4 b", k2=2, q4=4))
                cout = dramp.tile([NCORES * 128, 2 * NB], BF16, tag="agout",
                                  addr_space="Shared")
                nc.gpsimd.collective_compute(
                    "AllGather", mybir.AluOpType.bypass,
                    replica_groups=RG, ins=[cin[:]], outs=[cout[:]],
                )
                return cout

            def big_from_ag(cout, pool_lo, pool_hi, tag):
                """Gathered h -> two SBUF half tiles [128, 8*32], layout
                [p, kt*32+b] for kt 0..7 (lo) / 8..15 (hi).  Two DMAs on
                separate queues so kt 0..7 consumers start early."""
                lo = pool_lo.tile([128, 8 * NB], BF16, tag=tag + "lo")
                hi = pool_hi.tile([128, 8 * NB], BF16, tag=tag + "hi")
                nc.sync.dma_start(
                    lo[:].rearrange("p (k c) -> p k c", k=4),
                    cout[0:512].rearrange("(k p) c -> p k c", p=128))
                nc.sync.dma_start(
                    hi[:].rearrange("p (k c) -> p k c", k=4),
                    cout[512:1024].rearrange("(k p) c -> p k c", p=128))
                return (lo, hi)

            def big_block(bh, kt):
                lo, hi = bh
                t_ = lo if kt < 8 else hi
                k = kt % 8
                return t_[:, k * NB:k * NB + NB]

            def x2t_from_ag(cout, x2t, t):
                # write gathered h1 of step t into x2t's t-th 512 block
                nc.sync.dma_start(
                    x2t[:, t * 512:(t + 1) * 512]
                    .rearrange("p (k c) -> p k c", k=8),
                    cout.rearrange("(k p) c -> p k c", p=128))

            def x2t_block(x2t, kt, t):
                return x2t[:, t * 512 + kt * NB: t * 512 + kt * NB + NB]

            SIG = mybir.ActivationFunctionType.Sigmoid
            TANH = mybir.ActivationFunctionType.Tanh

            def cell(gsrc, add_ap, c_old, ltag, out_idx=None):
                """LSTM cell elementwise.  gsrc: [psg0, psg1] PSUM pair
                (bias already accumulated) or single [32, GL] AP.
                add_ap: optional [32, GL] AP added to gsrc (bulk-U term).
                Returns (c_new, ht), ht = bf16 block-transposed [32, SL]."""
                if isinstance(gsrc, (list, tuple)):
                    h0, h1 = gsrc[0][:], gsrc[1][:]
                else:
                    h0, h1 = gsrc[:, 0:512], gsrc[:, 512:GL]
                if add_ap is not None:
                    ga = gwp.tile([32, 512], F32, tag="ga")
                    gb = gwp.tile([32, 512], F32, tag="gb")
                    nc.vector.tensor_add(ga[:], h0, add_ap[:, 0:512])
                    nc.vector.tensor_add(gb[:], h1, add_ap[:, 512:GL])
                    h0, h1 = ga[:], gb[:]
                act = nc.scalar.activation
                if_t = gwp.tile([32, 2 * SL], F32, tag="ift")
                gt_t = gwp.tile([32, SL], F32, tag="gtt")
                o_t = gwp.tile([32, SL], F32, tag="ot")
                act(if_t[:], h0, SIG)                  # i, f fused
                act(gt_t[:], h1[:, 0:SL], TANH)
                act(o_t[:], h1[:, SL:2 * SL], SIG)
                tmp = gwp.tile([32, SL], F32, tag="tmp")
                nc.vector.tensor_mul(tmp[:], if_t[:, 0:SL], gt_t[:])
                c_new = cstp.tile([32, SL], F32, tag="c" + ltag)
                if c_old is not None:
                    cmul = gwp.tile([32, SL], F32, tag="cmul")
                    nc.gpsimd.tensor_mul(cmul[:], if_t[:, SL:2 * SL],
                                         c_old[:])
                    nc.vector.tensor_add(c_new[:], cmul[:], tmp[:])
                else:
                    nc.vector.tensor_copy(c_new[:], tmp[:])
                tanh_c = gwp.tile([32, SL], F32, tag="tanhc")
                act(tanh_c[:], c_new[:], TANH)
                hb = gwp.tile([32, SL], BF16, tag="hb")
                nc.vector.tensor_mul(hb[:], o_t[:], tanh_c[:])
                if out_idx is not None:
                    nc.sync.dma_start(chunks_out[out_idx], hb[:])
                ht = hslp.tile([32, SL], BF16, tag="hsl")
                nc.vector.transpose(ht[:], hb[:])
                return c_new, ht

            def bias_mm(psums, bias_t, m):
                """Accumulate bias into psums via K=1 matmul (opens the
                accumulation group: start=True)."""
                for n in range(2):
                    nc.tensor.matmul(
                        psums[n][:], ones_sb[0:1, 0:m],
                        bias_t[0:1, n * 512:(n + 1) * 512],
                        start=True, stop=False, skip_group_check=True)

            def bulk_u(lhs_fn, rhs_fn, bias_t):
                """U[t] = X[t] @ Wih_l^T + b for 4 steps, one M=128 pass.
                uts[1..3] are base-0 [32, GL] fp32 tiles; uts[0] None
                (use u_sb[0:32])."""
                psums = [psup.tile([128, 512], F32, tag="psu",
                                   name=f"psu{n_}") for n_ in range(2)]
                bias_mm(psums, bias_t, 128)
                for kt in range(NT):
                    rhs = rhs_fn(kt)
                    lhs = lhs_fn(kt)
                    for n in range(2):
                        nc.tensor.matmul(
                            psums[n][:], lhs, rhs[:, n * 512:(n + 1) * 512],
                            start=False, stop=(kt == NT - 1),
                            skip_group_check=True)
                u_sb = usbp.tile([128, GL], F32, tag="usb")
                for n in range(2):
                    nc.vector.tensor_copy(u_sb[:, n * 512:(n + 1) * 512],
                                          psums[n][:])
                uts = [None] * 4
                for t in range(1, 4):
                    ut = utp.tile([32, GL], F32, tag="ut")
                    nc.gpsimd.dma_start(ut[:], u_sb[32 * t:32 * t + 32, :])
                    uts[t] = ut
                return u_sb, uts

            def whh_matmuls(h_lhs_fn, whh_sb, extra=None, bias_t=None,
                            mid_hook=None):
                """K=2048 accumulation vs resident weights -> [psg0, psg1].
                extra: (lhs_fn, wih_sb) second K=2048 accumulation.
                bias_t: bias row accumulated first via K=1 matmul.
                mid_hook: emitted between the two accumulations (fills the
                tensor queue while extra's operand is still in flight)."""
                psums = [psgp.tile([32, 512], F32, tag="psg",
                                   name=f"psg{n_}") for n_ in range(2)]
                first = True
                if bias_t is not None:
                    bias_mm(psums, bias_t, 32)
                    first = False
                for kt in range(NT):
                    lhs = h_lhs_fn(kt)
                    for n in range(2):
                        nc.tensor.matmul(
                            psums[n][:], lhs,
                            whh_sb[:, kt * GL + n * 512:
                                   kt * GL + n * 512 + 512],
                            start=(first and kt == 0),
                            stop=(extra is None and kt == NT - 1),
                            skip_group_check=True)
                if mid_hook is not None:
                    mid_hook()
                if extra is not None:
                    lhs2, wih_sb = extra
                    for kt in range(NT):
                        lhs = lhs2(kt)
                        for n in range(2):
                            nc.tensor.matmul(
                                psums[n][:], lhs,
                                wih_sb[:, kt * GL + n * 512:
                                       kt * GL + n * 512 + 512],
                                start=False, stop=(kt == NT - 1),
                                skip_group_check=True)
                return psums

            def wih_matmuls(lhs_fn, wih_sb, bias_t):
                """K=2048 accumulation vs resident Wih only (layer-1 t0)."""
                psums = [psgp.tile([32, 512], F32, tag="psg",
                                   name=f"psgw{n_}") for n_ in range(2)]
                bias_mm(psums, bias_t, 32)
                for kt in range(NT):
                    lhs = lhs_fn(kt)
                    for n in range(2):
                        nc.tensor.matmul(
                            psums[n][:], lhs,
                            wih_sb[:, kt * GL + n * 512:
                                   kt * GL + n * 512 + 512],
                            start=False, stop=(kt == NT - 1),
                            skip_group_check=True)
                return psums

            def load_xin(x_dram, name):
                # host pre-laid-out [p, kt*128 + t*32 + b]; one linear DMA
                t_ = xinp.tile([128, NT * 128], BF16, tag="xin", name=name)
                nc.sync.dma_start(t_[:], x_dram[:, :])
                return t_

            def xin_block(x_sb, kt, t):
                return x_sb[:, kt * 128 + NB * t: kt * 128 + NB * t + NB]

            def dual_scan(l0_init_lhs, uts, u_sb, whh0, whh1, wih1, b1,
                          c1_init, c2_init, x2t_out, h2_init_big,
                          zero_init, store_de):
                """Wavefront over both layers: layer-0 (bulk-U + Whh0) and
                layer-1 (step-wise: Wih1 @ h1_t + Whh1 @ h2_{t-1}).
                Layer-1's matmuls fill layer-0's AllGather windows."""
                c1p, c2p = c1_init, c2_init
                h2_prev = h2_init_big
                for t in range(4):
                    # ---- layer 0 step t ----
                    if zero_init and t == 0:
                        c1p, ht = cell(u_sb[0:32, :], None, None, "1")
                    else:
                        if t == 0:
                            lhs = l0_init_lhs
                        else:
                            lhs = lambda kt: x2t_block(x2t_out, kt, t - 1)
                        psums = whh_matmuls(lhs, whh0)
                        if uts[t] is not None:
                            c1p, ht = cell(psums, uts[t][:], c1p, "1")
                        else:
                            c1p, ht = cell(psums, u_sb[0:32, :], c1p, "1")
                    cout = ag(ht)
                    x2t_from_ag(cout, x2t_out, t)
                    # ---- layer 1 step t ----
                    wih_lhs = lambda kt: x2t_block(x2t_out, kt, t)
                    if zero_init and t == 0:
                        psums = wih_matmuls(wih_lhs, wih1, b1)
                        c2p, ht = cell(psums, None, None, "2")
                    else:
                        h2b = h2_prev
                        psums = whh_matmuls(
                            lambda kt: big_block(h2b, kt), whh1,
                            extra=(wih_lhs, wih1), bias_t=b1)
                        c2p, ht = cell(
                            psums, None, c2p, "2",
                            out_idx=(0 if store_de and t == 3 else None))
                    cout = ag(ht)
                    h2_prev = big_from_ag(cout, h2lop, h2hip, "h2")
                return c1p, c2p, h2_prev

            # =========================================================
            # Phase E: encoder (batch 32 = [x2 fwd chain, x1-rev chain])
            # =========================================================
            xe_sb = load_xin(xET, "xe_sb")          # first on sync ring
            eb0 = load_bias(eB, 0, "eb0")
            eb1 = load_bias(eB, 1, "eb1")
            db0 = load_bias(dB, 0, "db0")
            db1 = load_bias(dB, 1, "db1")
            whh_e0 = load_w(eWhh, 0, "whh_e0", nc.sync)
            whh_e1 = load_w(eWhh, 1, "whh_e1", nc.sync)

            def estream(kt):
                wt = wstrp.tile([128, GL], BF16, tag="wstr")
                nc.scalar.dma_start(
                    wt[:], eWih[0, kt * 128:(kt + 1) * 128, :])
                return wt[:]

            u_sb, uts = bulk_u(
                lambda kt: xe_sb[:, kt * 128:(kt + 1) * 128], estream, eb0)
            wih_e1 = load_w(eWih, 1, "wih_e1", nc.scalar)
            whh_d0 = load_w(dWhh, 0, "whh_d0", nc.scalar)
            x2t_e = x2tp.tile([128, NT * 128], BF16, tag="x2t")
            c_e1, c_e2, h2_big = dual_scan(
                None, uts, u_sb, whh_e0, whh_e1, wih_e1, eb1,
                None, None, x2t_e, None, True, False)

            # =========================================================
            # Phase D1: decoder consume (batch = [x1 fwd, x2-rev])
            # =========================================================
            wih_d0 = load_w(dWih, 0, "wih_d0", nc.scalar)
            xd_sb = load_xin(xDT, "xd_sb")
            u_sb, uts = bulk_u(
                lambda kt: xd_sb[:, kt * 128:(kt + 1) * 128],
                lambda kt: wih_d0[:, kt * GL:(kt + 1) * GL], db0)
            whh_d1 = load_w(dWhh, 1, "whh_d1", nc.scalar)
            wih_d1 = load_w(dWih, 1, "wih_d1", nc.scalar)
            x2t_d = x2tp.tile([128, NT * 128], BF16, tag="x2t")
            c1, c2, h2_big = dual_scan(
                lambda kt: x2t_block(x2t_e, kt, 3), uts, u_sb,
                whh_d0, whh_d1, wih_d1, db1,
                c_e1, c_e2, x2t_d, h2_big, False, True)

            # =========================================================
            # Phase D2: autoregressive decoder (7 steps, zero streaming)
            # =========================================================
            conv_tiles = {"de": h2_big}
            h1_big = None
            cb_sb = biasp.tile([64, C], F32, tag="cb", name="cb_sb", bufs=1)
            nc.gpsimd.dma_start(cb_sb[:], cB[:])
            cwt_sb = cwtp.tile([128, 8 * C], BF16, tag="cwt")
            nc.scalar.dma_start(
                cwt_sb[:].rearrange("p (j o) -> p j o", j=8),
                cWT.rearrange("(j p) o -> p j o", p=128))

            def conv_hook(w):
                """One conv output column: 8 gathered lhs tiles (gpsimd
                copies) + 8 matmuls, accumulated into pcv, then bias +
                leaky-relu on gpsimd.  Issued inside the AR loop's tensor
                idle window."""
                b1 = [conv_tiles["de"], conv_tiles["ar0"],
                      conv_tiles["ar1"], conv_tiles["ar2"]]
                b2 = [conv_tiles["ar2"], conv_tiles["ar1"],
                      conv_tiles["ar0"], conv_tiles["de"]]

                def hook():
                    pcv = psup.tile([128, 512], F32, tag="psu",
                                    name=f"pcv{w}")
                    first = True
                    for br, src in ((0, b1[w]), (1, b2[w])):
                        lo, hi = src
                        for j in range(4):
                            st = gwp.tile([128, 64], BF16, tag="cvl",
                                          bufs=2, name=f"cvl{w}_{br}_{j}")
                            for half, tl in ((0, lo), (1, hi)):
                                nc.gpsimd.tensor_copy(
                                    st[:, half * 32:half * 32 + 32]
                                    .rearrange("p (h b) -> p h b", h=2),
                                    tl[:].rearrange("p (kt b) -> p kt b",
                                                    kt=8)
                                    [:, j::4, 16 * br:16 * br + 16])
                            nc.tensor.matmul(
                                pcv[0:64, :], st[:],
                                cwt_sb[:, (4 * br + j) * C:
                                       (4 * br + j + 1) * C],
                                start=first, stop=(br == 1 and j == 3),
                                skip_group_check=True)
                            first = False
                    cvs = gwp.tile([64, C], F32, tag="g", name=f"cvs{w}")
                    nc.vector.tensor_add(cvs[:], pcv[0:64, :], cb_sb[:])
                    cvo = gwp.tile([64, C], F32, tag="g2", name=f"cvo{w}")
                    nc.vector.tensor_scalar_mul(cvo[:], cvs[:], 0.2)
                    nc.vector.tensor_max(cvo[:], cvo[:], cvs[:])
                    nc.gpsimd.dma_start(convout[w], cvo[:])
                return hook

            for t in range(NAR):
                h2b, h1b = h2_big, h1_big
                if t == 0:
                    l0_lhs = lambda kt: x2t_block(x2t_d, kt, 3)
                else:
                    l0_lhs = lambda kt: big_block(h1b, kt)
                psums = whh_matmuls(
                    l0_lhs, whh_d0,
                    extra=(lambda kt: big_block(h2b, kt), wih_d0),
                    bias_t=db0)
                c1, ht = cell(psums, None, c1, "1")
                cout = ag(ht)
                h1_big = big_from_ag(cout, h1lop, h1hip, "h1")

                h1b2 = h1_big
                psums = whh_matmuls(
                    lambda kt: big_block(h2b, kt), whh_d1,
                    extra=(lambda kt: big_block(h1b2, kt), wih_d1),
                    bias_t=db1,
                    mid_hook=(conv_hook(t - 3) if t >= 3 else None))
                c2, ht = cell(psums, None, c2, "2", out_idx=t + 1)
                if t < NAR - 1:
                    cout = ag(ht)
                    h2_big = big_from_ag(cout, h2lop, h2hip, "h2")
                    if t < 3:
                        conv_tiles[f"ar{t}"] = h2_big

    nc.compile()
    return nc


def _prep_inputs(x1, x2, enc_Wih, enc_Whh, enc_bih, enc_bhh,
                 dec_Wih, dec_Whh, dec_bih, dec_bhh, conv_W, conv_b):
    def colvecs(x):
        return [np.ascontiguousarray(x[:, :, :, t].reshape(B, L))
                for t in range(4)]

    x1c, x2c = colvecs(x1), colvecs(x2)

    def ximg(xa):
        # [4, L, NB] -> SBUF image [128, kt*128 + t*32 + b]
        return np.ascontiguousarray(
            xa.reshape(4, NT, 128, NB).transpose(2, 1, 0, 3)
            .reshape(128, NT * 4 * NB)).astype(NPBF)

    xET = ximg(np.stack([
        np.concatenate([x2c[t], x1c[3 - t]], axis=0)[:, PERM].T
        for t in range(4)]))
    xDT = ximg(np.stack([
        np.concatenate([x1c[t], x2c[3 - t]], axis=0)[:, PERM].T
        for t in range(4)]))

    def prep_core(k, Wih, Whh, bih, bhh):
        rows = np.concatenate([g * L + PERM[k * SL:(k + 1) * SL]
                               for g in range(4)])
        wihT = np.stack([np.ascontiguousarray(Wih[l][rows][:, PERM].T)
                         for l in range(2)])
        whhT = np.stack([np.ascontiguousarray(Whh[l][rows][:, PERM].T)
                         for l in range(2)])
        bb = np.stack([(bih[l] + bhh[l])[rows] for l in range(2)])
        return (wihT.astype(NPBF), whhT.astype(NPBF),
                bb.reshape(2, 1, GL).astype(NPBF))

    cWT = np.ascontiguousarray(conv_W.T).astype(NPBF)
    cBr = np.broadcast_to(conv_b[None, :], (64, C)).copy().astype(np.float32)

    in_maps = []
    for k in range(NCORES):
        eWihT, eWhhT, eBr = prep_core(k, enc_Wih, enc_Whh, enc_bih, enc_bhh)
        dWihT, dWhhT, dBr = prep_core(k, dec_Wih, dec_Whh, dec_bih, dec_bhh)
        in_maps.append({
            "xET": xET, "xDT": xDT,
            "eWih": eWihT, "eWhh": eWhhT, "eB": eBr,
            "dWih": dWihT, "dWhh": dWhhT, "dB": dBr,
            "cWT": cWT, "cB": cBr,
        })
    return in_maps


def _postprocess(results, x1, x2):
    # gather chunk slices across cores: core k owns features [256k:256k+256)
    chunks = np.zeros((8, B * 2, L), np.float32)
    for k in range(NCORES):
        chunks[:, :, k * SL:(k + 1) * SL] = \
            results[k]["chunks_out"].astype(np.float32)
    convout = results[0]["convout"]

    def tochunk(t, half):
        v = chunks[t, half * B:(half + 1) * B, :]   # [16, L] dev order
        return v[:, IPERM].reshape(B, C, H)

    de1 = tochunk(0, 0)
    p1 = [tochunk(1 + j, 0) for j in range(NAR)]
    de2 = tochunk(0, 1)
    p2 = [tochunk(1 + j, 1) for j in range(NAR)]

    mid1 = np.stack([de1, p1[0], p1[1], p1[2]], axis=-1)
    tail1 = np.stack([p1[3], p1[4], p1[5], p1[6]], axis=-1)
    head2 = np.stack([p2[6], p2[5], p2[4], p2[3]], axis=-1)
    mid2 = np.stack([p2[2], p2[1], p2[0], de2], axis=-1)

    out = convout.reshape(4, 4, B, C).transpose(2, 3, 1, 0)
    out = np.ascontiguousarray(out, dtype=np.float32)
    return (out, np.asarray(x1), mid1, tail1, head2, mid2, np.asarray(x2))


def _run(in_maps, trace=False):
    if "nc" not in _CACHE:
        _CACHE["nc"] = _build_nc()
        _CACHE["tmpdir"] = tempfile.mkdtemp(prefix="lstmk_")
    nc = _CACHE["nc"]
    res = bass_utils.run_bass_kernel_spmd(
        nc, in_maps, core_ids=list(range(NCORES)), trace=trace,
        tmpdir=_CACHE["tmpdir"] if trace else None)
    return res


def kernel(**inputs):
    inputs = {k: np.asarray(v, dtype=np.float32) for k, v in inputs.items()}
    in_maps = _prep_inputs(**inputs)
    res = _run(in_maps, trace=False)
    return _postprocess(res.results, inputs["x1"], inputs["x2"])


def kernel_traced(**inputs):
    inputs = {k: np.asarray(v, dtype=np.float32) for k, v in inputs.items()}
    in_maps = _prep_inputs(**inputs)
    res = _run(in_maps, trace=True)
    return _postprocess(res.results, inputs["x1"], inputs["x2"]), res


# revision 31
# speedup vs baseline: 1.1055x; 1.0040x over previous
"""Trainium2 Bass kernel for the 2-layer LSTM encoder/decoder problem.

Strategy (8 NeuronCores):
  - Tensor-parallel shard of the 4L=8192 gate rows: core k owns rows
    [256k:256k+256) of each gate (i,f,g,o) -> 1024 gate rows / core.
  - Activations live transposed [feature, batch] on device; batch = 32
    (the two independent scan chains of the reference are batched).
  - Non-autoregressive phases are processed layer-by-layer: the Wih
    contribution for all 4 timesteps is one M=128 bulk matmul; only the
    Whh recurrence is stepwise, with Whh SBUF-resident.
  - Matmuls run in bf16 (PSUM accumulation fp32); cell state stays fp32.
  - Gate biases enter PSUM via K=1 matmuls (ones-vector x bias row), so
    the LSTM cell activations read PSUM directly with no DVE adds.
  - Hidden slices are AllGather'ed (bf16) between layer-steps in a
    [128 x 64] per-rank layout (128B DMA lines); the gathered tensor is
    read back as two half-tiles on separate DMA queues so dependent
    matmuls start after half the transfer.
  - A dummy warmup AllGather at kernel start absorbs the cold ncfw
    barrier into the initial weight-load window.
  - The 1x1-conv epilogue is interleaved into the autoregressive loop's
    tensor-engine idle windows (one output column per iteration).
"""

import tempfile

import numpy as np
import ml_dtypes

import concourse.bass as bass
import concourse.bacc as bacc
import concourse.mybir as mybir
import concourse.tile as tile
from concourse import bass_utils

# Problem constants (hardcoded per contract)
C, H, W = 512, 4, 4
SPLIT, PRED = 4, 4
L = 2048           # lstm feature size
B = 16             # reference batch
NB = 32            # device batch (two chains)
NCORES = 8
SL = L // NCORES   # 256: hidden slice per core
GL = 4 * SL        # 1024: gate rows per core
NT = L // 128      # 16 k-tiles
NAR = PRED + SPLIT - 1  # 7 autoregressive steps

F32 = mybir.dt.float32
BF16 = mybir.dt.bfloat16
NPBF = ml_dtypes.bfloat16

# Permutation: device feature f' = 512*h + c  <->  natural f = 4*c + h
PERM = np.array([4 * (f % C) + f // C for f in range(L)], dtype=np.int64)
IPERM = np.argsort(PERM)

_CACHE = {}


def _build_nc():
    nc = bacc.Bacc("TRN2", target_bir_lowering=False, debug=False,
                   num_devices=NCORES)

    def din(name, shape, dt=F32):
        return nc.dram_tensor(name, shape, dt, kind="ExternalInput").ap()

    def dout(name, shape, dt=F32):
        return nc.dram_tensor(name, shape, dt, kind="ExternalOutput").ap()

    xET = din("xET", [128, NT * 4 * NB], BF16)
    xDT = din("xDT", [128, NT * 4 * NB], BF16)
    eWih = din("eWih", [2, L, GL], BF16)
    eWhh = din("eWhh", [2, L, GL], BF16)
    dWih = din("dWih", [2, L, GL], BF16)
    dWhh = din("dWhh", [2, L, GL], BF16)
    eB = din("eB", [2, 1, GL], BF16)      # bias row (enters psum via matmul)
    dB = din("dB", [2, 1, GL], BF16)
    cWT = din("cWT", [2 * C, C], BF16)    # conv_W.T
    cB = din("cB", [64, C])               # conv bias replicated over 64 rows
    ident = din("ident", [32, 32], BF16)  # identity for PE transposes

    # per-core own h2 slice [32 batch, 256 feat] per chunk; host gathers
    chunks_out = dout("chunks_out", [8, NB, SL], BF16)
    convout = dout("convout", [4, 64, C])   # [w, (h,b), out_ch]

    with tile.TileContext(nc) as tc:
        with (
            tc.tile_pool(name="bias", bufs=4) as biasp,
            tc.tile_pool(name="whh", bufs=4) as whhp,
            tc.tile_pool(name="cwt", bufs=1) as cwtp,
            tc.tile_pool(name="wstr", bufs=3) as wstrp,
            tc.tile_pool(name="x2t", bufs=1) as x2tp,
            tc.tile_pool(name="xin", bufs=1) as xinp,
            tc.tile_pool(name="usb", bufs=1) as usbp,
            tc.tile_pool(name="ut", bufs=3) as utp,
            tc.tile_pool(name="h2q", bufs=7) as h2qp,
            tc.tile_pool(name="h1q", bufs=2) as h1qp,
            tc.tile_pool(name="gw", bufs=1) as gwp,
            tc.tile_pool(name="cst", bufs=2) as cstp,
            tc.tile_pool(name="hsl", bufs=2) as hslp,
            tc.tile_pool(name="psu", bufs=2, space="PSUM") as psup,
            tc.tile_pool(name="psg", bufs=4, space="PSUM") as psgp,
            tc.tile_pool(name="pst", bufs=2, space="PSUM") as pstp,
            tc.tile_pool(name="dram", bufs=3, space="DRAM") as dramp,
        ):
            RG = [list(range(NCORES))]

            # ones row for bias-into-psum matmuls
            ones_sb = biasp.tile([1, 128], BF16, tag="ones", bufs=1)
            nc.gpsimd.memset(ones_sb[:], 1.0)
            ident_sb = biasp.tile([32, 32], BF16, tag="ident", bufs=1)
            nc.gpsimd.dma_start(ident_sb[:], ident[:, :])

            def load_bias(src, l, name):
                t_ = biasp.tile([1, GL], BF16, tag="bias", name=name)
                nc.gpsimd.dma_start(t_[:], src[l])
                return t_

            def load_w(w_dram, l, name, eng):
                # resident weight matrix -> [128, NT*GL] layout [p, kt*GL+n]
                wt = whhp.tile([128, NT * GL], BF16, tag="whh", name=name)
                eng.dma_start(
                    wt[:].rearrange("p (kt n) -> p kt n", kt=NT),
                    w_dram[l].rearrange("(kt p) n -> p kt n", p=128),
                )
                return wt

            def ag(ht):
                """AllGather this core's h slice.  ht: [128, 64] SBUF tile
                in feature-partition layout (ht[p, 32*k2+b] =
                h[b, 128*k2+p]), so the cin write is fully contiguous and
                gathered rank blocks give 128B lines on readback."""
                cin = dramp.tile([128, 2 * NB], BF16, tag="agin")
                nc.gpsimd.dma_start(cin[:, :], ht[:])
                cout = dramp.tile([NCORES * 128, 2 * NB], BF16, tag="agout",
                                  addr_space="Shared")
                nc.gpsimd.collective_compute(
                    "AllGather", mybir.AluOpType.bypass,
                    replica_groups=RG, ins=[cin[:]], outs=[cout[:]],
                )
                return cout

            def big_from_ag(cout, pool, tag, engs):
                """Gathered h -> four SBUF quarter tiles [128, 4*32],
                layout [p, kt*32+b] for kt 4q..4q+3.  Separate DMAs
                (queues per engs) so early-kt consumers start first."""
                qs = []
                for q in range(4):
                    t_ = pool.tile([128, 4 * NB], BF16, tag=f"{tag}q{q}",
                                   name=f"{tag}q{q}")
                    engs[q].dma_start(
                        t_[:].rearrange("p (k c) -> p k c", k=2),
                        cout[q * 256:(q + 1) * 256]
                        .rearrange("(k p) c -> p k c", p=128))
                    qs.append(t_)
                return qs

            def big_block(bh, kt):
                return bh[kt // 4][:, (kt % 4) * NB:(kt % 4) * NB + NB]

            def x2t_from_ag(cout, x2t, t):
                # write gathered h1 of step t into the two x2t half tiles
                xa, xb = x2t
                for half, xt in ((0, xa), (1, xb)):
                    nc.sync.dma_start(
                        xt[:, t * 256:(t + 1) * 256]
                        .rearrange("p (k c) -> p k c", k=4),
                        cout[half * 512:(half + 1) * 512]
                        .rearrange("(k p) c -> p k c", p=128))

            def x2t_block(x2t, kt, t):
                xt = x2t[0] if kt < 8 else x2t[1]
                k = kt % 8
                return xt[:, t * 256 + k * NB: t * 256 + k * NB + NB]

            SIG = mybir.ActivationFunctionType.Sigmoid
            TANH = mybir.ActivationFunctionType.Tanh

            def cell(gsrc, add_ap, c_old, ltag, out_idx=None):
                """LSTM cell elementwise.  gsrc: [psg0, psg1] PSUM pair
                (bias already accumulated) or single [32, GL] AP.
                add_ap: optional [32, GL] AP added to gsrc (bulk-U term).
                Returns (c_new, ht), ht = bf16 block-transposed [32, SL]."""
                if isinstance(gsrc, (list, tuple)):
                    h0, h1 = gsrc[0][:], gsrc[1][:]
                else:
                    h0, h1 = gsrc[:, 0:512], gsrc[:, 512:GL]
                if add_ap is not None:
                    ga = gwp.tile([32, 512], F32, tag="ga")
                    gb = gwp.tile([32, 512], F32, tag="gb")
                    nc.vector.tensor_add(ga[:], h0, add_ap[:, 0:512])
                    nc.vector.tensor_add(gb[:], h1, add_ap[:, 512:GL])
                    h0, h1 = ga[:], gb[:]
                act = nc.scalar.activation
                if_t = gwp.tile([32, 2 * SL], F32, tag="ift")
                gt_t = gwp.tile([32, SL], F32, tag="gtt")
                o_t = gwp.tile([32, SL], F32, tag="ot")
                act(if_t[:], h0, SIG)                  # i, f fused
                act(gt_t[:], h1[:, 0:SL], TANH)
                act(o_t[:], h1[:, SL:2 * SL], SIG)
                tmp = gwp.tile([32, SL], F32, tag="tmp")
                nc.vector.tensor_mul(tmp[:], if_t[:, 0:SL], gt_t[:])
                c_new = cstp.tile([32, SL], F32, tag="c" + ltag)
                if c_old is not None:
                    cmul = gwp.tile([32, SL], F32, tag="cmul")
                    nc.gpsimd.tensor_mul(cmul[:], if_t[:, SL:2 * SL],
                                         c_old[:])
                    nc.vector.tensor_add(c_new[:], cmul[:], tmp[:])
                else:
                    nc.vector.tensor_copy(c_new[:], tmp[:])
                tanh_c = gwp.tile([32, SL], F32, tag="tanhc")
                act(tanh_c[:], c_new[:], TANH)
                hb = gwp.tile([32, SL], BF16, tag="hb")
                nc.vector.tensor_mul(hb[:], o_t[:], tanh_c[:])
                if out_idx is not None:
                    nc.sync.dma_start(chunks_out[out_idx], hb[:])
                # PE transpose to feature-partition layout; cheap on the
                # tensor queue (sits between this step's matmuls and the
                # next step's covered Whh half)
                pst = pstp.tile([128, 2 * NB], BF16, tag="pst")
                for k2 in range(2):
                    nc.tensor.matmul(
                        pst[:, k2 * NB:(k2 + 1) * NB],
                        hb[:, k2 * 128:(k2 + 1) * 128], ident_sb[:],
                        is_transpose=True, start=True, stop=True,
                        skip_group_check=True)
                ht = hslp.tile([128, 2 * NB], BF16, tag="hsl")
                nc.vector.tensor_copy(ht[:], pst[:])
                return c_new, ht

            def bias_mm(psums, bias_t, m):
                """Accumulate bias into psums via K=1 matmul (opens the
                accumulation group: start=True)."""
                for n in range(2):
                    nc.tensor.matmul(
                        psums[n][:], ones_sb[0:1, 0:m],
                        bias_t[0:1, n * 512:(n + 1) * 512],
                        start=True, stop=False, skip_group_check=True)

            def bulk_u(lhs_fn, rhs_fn, bias_t):
                """U[t] = X[t] @ Wih_l^T + b for 4 steps, one M=128 pass.
                uts[1..3] are base-0 [32, GL] fp32 tiles; uts[0] None
                (use u_sb[0:32])."""
                psums = [psup.tile([128, 512], F32, tag="psu",
                                   name=f"psu{n_}") for n_ in range(2)]
                bias_mm(psums, bias_t, 128)
                for kt in range(NT):
                    rhs = rhs_fn(kt)
                    lhs = lhs_fn(kt)
                    for n in range(2):
                        nc.tensor.matmul(
                            psums[n][:], lhs, rhs[:, n * 512:(n + 1) * 512],
                            start=False, stop=(kt == NT - 1),
                            skip_group_check=True)
                u_sb = usbp.tile([128, GL], F32, tag="usb")
                for n in range(2):
                    nc.vector.tensor_copy(u_sb[:, n * 512:(n + 1) * 512],
                                          psums[n][:])
                uts = [None] * 4
                for t in range(1, 4):
                    ut = utp.tile([32, GL], F32, tag="ut")
                    nc.gpsimd.dma_start(ut[:], u_sb[32 * t:32 * t + 32, :])
                    uts[t] = ut
                return u_sb, uts

            def whh_matmuls(h_lhs_fn, whh_sb, extra=None, bias_t=None,
                            mid_hook=None):
                """K=2048 accumulation vs resident weights -> [psg0, psg1].
                extra: (lhs_fn, wih_sb) second K=2048 accumulation.
                bias_t: bias row accumulated first via K=1 matmul.
                mid_hook: emitted between the two accumulations (fills the
                tensor queue while extra's operand is still in flight)."""
                psums = [psgp.tile([32, 512], F32, tag="psg",
                                   name=f"psg{n_}") for n_ in range(2)]
                first = True
                if bias_t is not None:
                    bias_mm(psums, bias_t, 32)
                    first = False
                # n-major order: psg0 (i,f gates) stops before psg1, so
                # the cell's sigmoid + f*c start under the g,o matmuls
                for n in range(2):
                    for kt in range(NT):
                        nc.tensor.matmul(
                            psums[n][:], h_lhs_fn(kt),
                            whh_sb[:, kt * GL + n * 512:
                                   kt * GL + n * 512 + 512],
                            start=(first and kt == 0),
                            stop=(extra is None and kt == NT - 1),
                            skip_group_check=True)
                if mid_hook is not None:
                    mid_hook()
                if extra is not None:
                    lhs2, wih_sb = extra
                    for n in range(2):
                        for kt in range(NT):
                            nc.tensor.matmul(
                                psums[n][:], lhs2(kt),
                                wih_sb[:, kt * GL + n * 512:
                                       kt * GL + n * 512 + 512],
                                start=False, stop=(kt == NT - 1),
                                skip_group_check=True)
                return psums

            def wih_matmuls(lhs_fn, wih_sb, bias_t):
                """K=2048 accumulation vs resident Wih only (layer-1 t0)."""
                psums = [psgp.tile([32, 512], F32, tag="psg",
                                   name=f"psgw{n_}") for n_ in range(2)]
                bias_mm(psums, bias_t, 32)
                for n in range(2):
                    for kt in range(NT):
                        nc.tensor.matmul(
                            psums[n][:], lhs_fn(kt),
                            wih_sb[:, kt * GL + n * 512:
                                   kt * GL + n * 512 + 512],
                            start=False, stop=(kt == NT - 1),
                            skip_group_check=True)
                return psums

            def load_xin(x_dram, name):
                # host pre-laid-out [p, kt*128 + t*32 + b]; one linear DMA
                t_ = xinp.tile([128, NT * 128], BF16, tag="xin", name=name)
                nc.sync.dma_start(t_[:], x_dram[:, :])
                return t_

            def xin_block(x_sb, kt, t):
                return x_sb[:, kt * 128 + NB * t: kt * 128 + NB * t + NB]

            def dual_scan(l0_init_lhs, uts, u_sb, whh0, whh1, wih1, b1,
                          c1_init, c2_init, x2t_out, h2_init_big,
                          zero_init, store_de):
                """Wavefront over both layers: layer-0 (bulk-U + Whh0) and
                layer-1 (step-wise: Wih1 @ h1_t + Whh1 @ h2_{t-1}).
                Layer-1's matmuls fill layer-0's AllGather windows."""
                c1p, c2p = c1_init, c2_init
                h2_prev = h2_init_big
                for t in range(4):
                    # ---- layer 0 step t ----
                    if zero_init and t == 0:
                        c1p, ht = cell(u_sb[0:32, :], None, None, "1")
                    else:
                        if t == 0:
                            lhs = l0_init_lhs
                        else:
                            lhs = lambda kt: x2t_block(x2t_out, kt, t - 1)
                        psums = whh_matmuls(lhs, whh0)
                        if uts[t] is not None:
                            c1p, ht = cell(psums, uts[t][:], c1p, "1")
                        else:
                            c1p, ht = cell(psums, u_sb[0:32, :], c1p, "1")
                    cout = ag(ht)
                    x2t_from_ag(cout, x2t_out, t)
                    # ---- layer 1 step t ----
                    wih_lhs = lambda kt: x2t_block(x2t_out, kt, t)
                    if zero_init and t == 0:
                        psums = wih_matmuls(wih_lhs, wih1, b1)
                        c2p, ht = cell(psums, None, None, "2")
                    else:
                        h2b = h2_prev
                        psums = whh_matmuls(
                            lambda kt: big_block(h2b, kt), whh1,
                            extra=(wih_lhs, wih1), bias_t=b1)
                        c2p, ht = cell(
                            psums, None, c2p, "2",
                            out_idx=(0 if store_de and t == 3 else None))
                    cout = ag(ht)
                    h2_prev = big_from_ag(cout, h2qp, "h2",
                                          [nc.sync] * 4)
                return c1p, c2p, h2_prev

            # =========================================================
            # Phase E: encoder (batch 32 = [x2 fwd chain, x1-rev chain])
            # =========================================================
            xe_sb = load_xin(xET, "xe_sb")          # first on sync ring
            eb0 = load_bias(eB, 0, "eb0")
            eb1 = load_bias(eB, 1, "eb1")
            db0 = load_bias(dB, 0, "db0")
            db1 = load_bias(dB, 1, "db1")
            whh_e0 = load_w(eWhh, 0, "whh_e0", nc.sync)
            whh_e1 = load_w(eWhh, 1, "whh_e1", nc.sync)

            def estream(kt):
                wt = wstrp.tile([128, GL], BF16, tag="wstr")
                nc.scalar.dma_start(
                    wt[:], eWih[0, kt * 128:(kt + 1) * 128, :])
                return wt[:]

            u_sb, uts = bulk_u(
                lambda kt: xe_sb[:, kt * 128:(kt + 1) * 128], estream, eb0)
            wih_e1 = load_w(eWih, 1, "wih_e1", nc.scalar)
            whh_d0 = load_w(dWhh, 0, "whh_d0", nc.scalar)
            x2t_e = (x2tp.tile([128, 4 * 256], BF16, tag="x2ta",
                               name="x2t_ea"),
                     x2tp.tile([128, 4 * 256], BF16, tag="x2tb",
                               name="x2t_eb"))
            c_e1, c_e2, h2_big = dual_scan(
                None, uts, u_sb, whh_e0, whh_e1, wih_e1, eb1,
                None, None, x2t_e, None, True, False)

            # =========================================================
            # Phase D1: decoder consume (batch = [x1 fwd, x2-rev])
            # =========================================================
            wih_d0 = load_w(dWih, 0, "wih_d0", nc.scalar)
            xd_sb = load_xin(xDT, "xd_sb")
            cb_sb = biasp.tile([64, C], F32, tag="cb", name="cb_sb", bufs=1)
            nc.gpsimd.dma_start(cb_sb[:], cB[:])
            cwt_sb = cwtp.tile([128, 8 * C], BF16, tag="cwt")
            nc.gpsimd.dma_start(
                cwt_sb[:].rearrange("p (j o) -> p j o", j=8),
                cWT.rearrange("(j p) o -> p j o", p=128))
            u_sb, uts = bulk_u(
                lambda kt: xd_sb[:, kt * 128:(kt + 1) * 128],
                lambda kt: wih_d0[:, kt * GL:(kt + 1) * GL], db0)
            whh_d1 = load_w(dWhh, 1, "whh_d1", nc.scalar)
            wih_d1 = load_w(dWih, 1, "wih_d1", nc.scalar)
            x2t_d = (x2tp.tile([128, 4 * 256], BF16, tag="x2ta",
                               name="x2t_da"),
                     x2tp.tile([128, 4 * 256], BF16, tag="x2tb",
                               name="x2t_db"))
            c1, c2, h2_big = dual_scan(
                lambda kt: x2t_block(x2t_e, kt, 3), uts, u_sb,
                whh_d0, whh_d1, wih_d1, db1,
                c_e1, c_e2, x2t_d, h2_big, False, True)

            # =========================================================
            # Phase D2: autoregressive decoder (7 steps, zero streaming)
            # =========================================================
            conv_tiles = {"de": h2_big}
            h1_big = None
            ENGS4 = [nc.sync, nc.scalar, nc.sync, nc.scalar]

            def conv_hook(w):
                """One conv output column: 8 gathered lhs tiles (gpsimd
                copies) + 8 matmuls, accumulated into pcv, then bias +
                leaky-relu on gpsimd.  Issued inside the AR loop's tensor
                idle window."""
                b1 = [conv_tiles["de"], conv_tiles["ar0"],
                      conv_tiles["ar1"], conv_tiles["ar2"]]
                b2 = [conv_tiles["ar2"], conv_tiles["ar1"],
                      conv_tiles["ar0"], conv_tiles["de"]]

                def hook():
                    pcv = psup.tile([128, 512], F32, tag="psu",
                                    name=f"pcv{w}")
                    first = True
                    for br, src in ((0, b1[w]), (1, b2[w])):
                        for j in range(4):
                            st = gwp.tile([128, 64], BF16, tag="cvl",
                                          bufs=2, name=f"cvl{w}_{br}_{j}")
                            for h in range(4):
                                # global kt = j + 4h lives in quarter h
                                nc.gpsimd.tensor_copy(
                                    st[:, h * 16:h * 16 + 16],
                                    src[h][:, j * NB + 16 * br:
                                           j * NB + 16 * br + 16])
                            nc.tensor.matmul(
                                pcv[0:64, :], st[:],
                                cwt_sb[:, (4 * br + j) * C:
                                       (4 * br + j + 1) * C],
                                start=first, stop=(br == 1 and j == 3),
                                skip_group_check=True)
                            first = False
                    cvs = gwp.tile([64, C], F32, tag="g", name=f"cvs{w}")
                    nc.vector.tensor_add(cvs[:], pcv[0:64, :], cb_sb[:])
                    cvo = gwp.tile([64, C], F32, tag="g2", name=f"cvo{w}")
                    nc.vector.tensor_scalar_mul(cvo[:], cvs[:], 0.2)
                    nc.vector.tensor_max(cvo[:], cvo[:], cvs[:])
                    nc.gpsimd.dma_start(convout[w], cvo[:])
                return hook

            for t in range(NAR):
                h2b, h1b = h2_big, h1_big
                if t == 0:
                    l0_lhs = lambda kt: x2t_block(x2t_d, kt, 3)
                else:
                    l0_lhs = lambda kt: big_block(h1b, kt)
                psums = whh_matmuls(
                    l0_lhs, whh_d0,
                    extra=(lambda kt: big_block(h2b, kt), wih_d0),
                    bias_t=db0)
                c1, ht = cell(psums, None, c1, "1")
                cout = ag(ht)
                h1_big = big_from_ag(cout, h1qp, "h1", ENGS4)

                h1b2 = h1_big
                psums = whh_matmuls(
                    lambda kt: big_block(h2b, kt), whh_d1,
                    extra=(lambda kt: big_block(h1b2, kt), wih_d1),
                    bias_t=db1,
                    mid_hook=(conv_hook(t - 3) if t >= 3 else None))
                c2, ht = cell(psums, None, c2, "2", out_idx=t + 1)
                if t < NAR - 1:
                    cout = ag(ht)
                    h2_big = big_from_ag(cout, h2qp, "h2", ENGS4)
                    if t < 3:
                        conv_tiles[f"ar{t}"] = h2_big

    nc.compile()
    return nc


def _prep_inputs(x1, x2, enc_Wih, enc_Whh, enc_bih, enc_bhh,
                 dec_Wih, dec_Whh, dec_bih, dec_bhh, conv_W, conv_b):
    def colvecs(x):
        return [np.ascontiguousarray(x[:, :, :, t].reshape(B, L))
                for t in range(4)]

    x1c, x2c = colvecs(x1), colvecs(x2)

    def ximg(xa):
        # [4, L, NB] -> SBUF image [128, kt*128 + t*32 + b]
        return np.ascontiguousarray(
            xa.reshape(4, NT, 128, NB).transpose(2, 1, 0, 3)
            .reshape(128, NT * 4 * NB)).astype(NPBF)

    xET = ximg(np.stack([
        np.concatenate([x2c[t], x1c[3 - t]], axis=0)[:, PERM].T
        for t in range(4)]))
    xDT = ximg(np.stack([
        np.concatenate([x1c[t], x2c[3 - t]], axis=0)[:, PERM].T
        for t in range(4)]))

    def prep_core(k, Wih, Whh, bih, bhh):
        rows = np.concatenate([g * L + PERM[k * SL:(k + 1) * SL]
                               for g in range(4)])
        wihT = np.stack([np.ascontiguousarray(Wih[l][rows][:, PERM].T)
                         for l in range(2)])
        whhT = np.stack([np.ascontiguousarray(Whh[l][rows][:, PERM].T)
                         for l in range(2)])
        bb = np.stack([(bih[l] + bhh[l])[rows] for l in range(2)])
        return (wihT.astype(NPBF), whhT.astype(NPBF),
                bb.reshape(2, 1, GL).astype(NPBF))

    cWT = np.ascontiguousarray(conv_W.T).astype(NPBF)
    cBr = np.broadcast_to(conv_b[None, :], (64, C)).copy().astype(np.float32)
    ident = np.eye(32, dtype=NPBF)

    in_maps = []
    for k in range(NCORES):
        eWihT, eWhhT, eBr = prep_core(k, enc_Wih, enc_Whh, enc_bih, enc_bhh)
        dWihT, dWhhT, dBr = prep_core(k, dec_Wih, dec_Whh, dec_bih, dec_bhh)
        in_maps.append({
            "xET": xET, "xDT": xDT,
            "eWih": eWihT, "eWhh": eWhhT, "eB": eBr,
            "dWih": dWihT, "dWhh": dWhhT, "dB": dBr,
            "cWT": cWT, "cB": cBr, "ident": ident,
        })
    return in_maps


def _postprocess(results, x1, x2):
    # gather chunk slices across cores: core k owns features [256k:256k+256)
    chunks = np.zeros((8, B * 2, L), np.float32)
    for k in range(NCORES):
        chunks[:, :, k * SL:(k + 1) * SL] = \
            results[k]["chunks_out"].astype(np.float32)
    convout = results[0]["convout"]

    def tochunk(t, half):
        v = chunks[t, half * B:(half + 1) * B, :]   # [16, L] dev order
        return v[:, IPERM].reshape(B, C, H)

    de1 = tochunk(0, 0)
    p1 = [tochunk(1 + j, 0) for j in range(NAR)]
    de2 = tochunk(0, 1)
    p2 = [tochunk(1 + j, 1) for j in range(NAR)]

    mid1 = np.stack([de1, p1[0], p1[1], p1[2]], axis=-1)
    tail1 = np.stack([p1[3], p1[4], p1[5], p1[6]], axis=-1)
    head2 = np.stack([p2[6], p2[5], p2[4], p2[3]], axis=-1)
    mid2 = np.stack([p2[2], p2[1], p2[0], de2], axis=-1)

    out = convout.reshape(4, 4, B, C).transpose(2, 3, 1, 0)
    out = np.ascontiguousarray(out, dtype=np.float32)
    return (out, np.asarray(x1), mid1, tail1, head2, mid2, np.asarray(x2))


def _run(in_maps, trace=False):
    if "nc" not in _CACHE:
        _CACHE["nc"] = _build_nc()
        _CACHE["tmpdir"] = tempfile.mkdtemp(prefix="lstmk_")
    nc = _CACHE["nc"]
    res = bass_utils.run_bass_kernel_spmd(
        nc, in_maps, core_ids=list(range(NCORES)), trace=trace,
        tmpdir=_CACHE["tmpdir"] if trace else None)
    return res


def kernel(**inputs):
    inputs = {k: np.asarray(v, dtype=np.float32) for k, v in inputs.items()}
    in_maps = _prep_inputs(**inputs)
    res = _run(in_maps, trace=False)
    return _postprocess(res.results, inputs["x1"], inputs["x2"])


def kernel_traced(**inputs):
    inputs = {k: np.asarray(v, dtype=np.float32) for k, v in inputs.items()}
    in_maps = _prep_inputs(**inputs)
    res = _run(in_maps, trace=True)
    return _postprocess(res.results, inputs["x1"], inputs["x2"]), res


# revision 36
# speedup vs baseline: 1.1617x; 1.0508x over previous
"""Trainium2 Bass kernel for the 2-layer LSTM encoder/decoder problem.

Strategy (8 NeuronCores):
  - Tensor-parallel shard of the 4L=8192 gate rows: core k owns rows
    [256k:256k+256) of each gate (i,f,g,o) -> 1024 gate rows / core.
  - Activations live transposed [feature, batch] on device; batch = 32
    (the two independent scan chains of the reference are batched).
  - Non-autoregressive phases are processed layer-by-layer: the Wih
    contribution for all 4 timesteps is one M=128 bulk matmul; only the
    Whh recurrence is stepwise, with Whh SBUF-resident.
  - Matmuls run in bf16 (PSUM accumulation fp32); cell state stays fp32.
  - Gate biases enter PSUM via K=1 matmuls (ones-vector x bias row), so
    the LSTM cell activations read PSUM directly with no DVE adds.
  - Hidden slices are AllGather'ed (bf16) between layer-steps in a
    [128 x 64] per-rank layout (128B DMA lines); the gathered tensor is
    read back as two half-tiles on separate DMA queues so dependent
    matmuls start after half the transfer.
  - A dummy warmup AllGather at kernel start absorbs the cold ncfw
    barrier into the initial weight-load window.
  - The 1x1-conv epilogue is interleaved into the autoregressive loop's
    tensor-engine idle windows (one output column per iteration).
"""

import tempfile

import numpy as np
import ml_dtypes

import concourse.bass as bass
import concourse.bacc as bacc
import concourse.mybir as mybir
import concourse.tile as tile
from concourse import bass_utils

# Problem constants (hardcoded per contract)
C, H, W = 512, 4, 4
SPLIT, PRED = 4, 4
L = 2048           # lstm feature size
B = 16             # reference batch
NB = 32            # device batch (two chains)
NCORES = 8
SL = L // NCORES   # 256: hidden slice per core
GL = 4 * SL        # 1024: gate rows per core
NT = L // 128      # 16 k-tiles
NAR = PRED + SPLIT - 1  # 7 autoregressive steps

F32 = mybir.dt.float32
BF16 = mybir.dt.bfloat16
NPBF = ml_dtypes.bfloat16

# Permutation: device feature f' = 512*h + c  <->  natural f = 4*c + h
PERM = np.array([4 * (f % C) + f // C for f in range(L)], dtype=np.int64)
IPERM = np.argsort(PERM)

_CACHE = {}


def _build_nc():
    nc = bacc.Bacc("TRN2", target_bir_lowering=False, debug=False,
                   num_devices=NCORES)

    def din(name, shape, dt=F32):
        return nc.dram_tensor(name, shape, dt, kind="ExternalInput").ap()

    def dout(name, shape, dt=F32):
        return nc.dram_tensor(name, shape, dt, kind="ExternalOutput").ap()

    xET = din("xET", [128, NT * 4 * NB], BF16)
    xDT = din("xDT", [128, NT * 4 * NB], BF16)
    eWih = din("eWih", [2, L, GL], BF16)
    eWhh = din("eWhh", [2, L, GL], BF16)
    dWih = din("dWih", [2, L, GL], BF16)
    dWhh = din("dWhh", [2, L, GL], BF16)
    eB = din("eB", [2, 1, GL], BF16)      # bias row (enters psum via matmul)
    dB = din("dB", [2, 1, GL], BF16)
    cWT = din("cWT", [2 * C, C], BF16)    # conv_W.T
    cB = din("cB", [64, C])               # conv bias replicated over 64 rows
    ident = din("ident", [32, 32], BF16)  # identity for PE transposes

    # per-core own h2 slice [32 batch, 256 feat] per chunk; host gathers
    chunks_out = dout("chunks_out", [8, NB, SL], BF16)
    convout = dout("convout", [4, 64, C])   # [w, (h,b), out_ch]

    with tile.TileContext(nc) as tc:
        with (
            tc.tile_pool(name="bias", bufs=4) as biasp,
            tc.tile_pool(name="whh", bufs=4) as whhp,
            tc.tile_pool(name="cwt", bufs=1) as cwtp,
            tc.tile_pool(name="wstr", bufs=3) as wstrp,
            tc.tile_pool(name="x2t", bufs=1) as x2tp,
            tc.tile_pool(name="xin", bufs=1) as xinp,
            tc.tile_pool(name="usb", bufs=2) as usbp,
            tc.tile_pool(name="h2q", bufs=7) as h2qp,
            tc.tile_pool(name="h1q", bufs=2) as h1qp,
            tc.tile_pool(name="gw", bufs=1) as gwp,
            tc.tile_pool(name="cst", bufs=2) as cstp,
            tc.tile_pool(name="hsl", bufs=2) as hslp,
            tc.tile_pool(name="psu", bufs=2, space="PSUM") as psup,
            tc.tile_pool(name="psg", bufs=4, space="PSUM") as psgp,
            tc.tile_pool(name="pst", bufs=2, space="PSUM") as pstp,
            tc.tile_pool(name="dram", bufs=3, space="DRAM") as dramp,
        ):
            RG = [list(range(NCORES))]

            # ones row for bias-into-psum matmuls
            ones_sb = biasp.tile([1, 128], BF16, tag="ones", bufs=1)
            nc.gpsimd.memset(ones_sb[:], 1.0)
            ident_sb = biasp.tile([32, 32], BF16, tag="ident", bufs=1)
            nc.gpsimd.dma_start(ident_sb[:], ident[:, :])

            def load_bias(src, l, name):
                t_ = biasp.tile([1, GL], BF16, tag="bias", name=name)
                nc.gpsimd.dma_start(t_[:], src[l])
                return t_

            def load_w(w_dram, l, name, eng):
                # resident weight matrix -> [128, NT*GL] layout [p, kt*GL+n]
                wt = whhp.tile([128, NT * GL], BF16, tag="whh", name=name)
                eng.dma_start(
                    wt[:].rearrange("p (kt n) -> p kt n", kt=NT),
                    w_dram[l].rearrange("(kt p) n -> p kt n", p=128),
                )
                return wt

            def ag(ht):
                """AllGather this core's h slice.  ht: [128, 64] SBUF tile
                in feature-partition layout (ht[p, 32*k2+b] =
                h[b, 128*k2+p]), so the cin write is fully contiguous and
                gathered rank blocks give 128B lines on readback."""
                cin = dramp.tile([128, 2 * NB], BF16, tag="agin")
                nc.gpsimd.dma_start(cin[:, :], ht[:])
                cout = dramp.tile([NCORES * 128, 2 * NB], BF16, tag="agout",
                                  addr_space="Shared")
                nc.gpsimd.collective_compute(
                    "AllGather", mybir.AluOpType.bypass,
                    replica_groups=RG, ins=[cin[:]], outs=[cout[:]],
                )
                return cout

            def big_from_ag(cout, pool, tag, engs):
                """Gathered h -> four SBUF quarter tiles [128, 4*32],
                layout [p, kt*32+b] for kt 4q..4q+3.  Separate DMAs
                (queues per engs) so early-kt consumers start first."""
                qs = []
                for q in range(4):
                    t_ = pool.tile([128, 4 * NB], BF16, tag=f"{tag}q{q}",
                                   name=f"{tag}q{q}")
                    engs[q].dma_start(
                        t_[:].rearrange("p (k c) -> p k c", k=2),
                        cout[q * 256:(q + 1) * 256]
                        .rearrange("(k p) c -> p k c", p=128))
                    qs.append(t_)
                return qs

            def big_block(bh, kt):
                return bh[kt // 4][:, (kt % 4) * NB:(kt % 4) * NB + NB]

            def x2t_from_ag(cout, x2t, t):
                # write gathered h1 of step t into the two x2t half tiles
                xa, xb = x2t
                for half, xt in ((0, xa), (1, xb)):
                    nc.sync.dma_start(
                        xt[:, t * 256:(t + 1) * 256]
                        .rearrange("p (k c) -> p k c", k=4),
                        cout[half * 512:(half + 1) * 512]
                        .rearrange("(k p) c -> p k c", p=128))

            def x2t_block(x2t, kt, t):
                xt = x2t[0] if kt < 8 else x2t[1]
                k = kt % 8
                return xt[:, t * 256 + k * NB: t * 256 + k * NB + NB]

            SIG = mybir.ActivationFunctionType.Sigmoid
            TANH = mybir.ActivationFunctionType.Tanh

            def cell(gsrc, add_ap, c_old, ltag, out_idx=None):
                """LSTM cell elementwise.  gsrc: [psg0, psg1] PSUM pair
                (bias already accumulated) or single [32, GL] AP.
                add_ap: optional [32, GL] AP added to gsrc (bulk-U term).
                Returns (c_new, ht), ht = bf16 block-transposed [32, SL]."""
                if isinstance(gsrc, (list, tuple)):
                    h0, h1 = gsrc[0][:], gsrc[1][:]
                else:
                    h0, h1 = gsrc[:, 0:512], gsrc[:, 512:GL]
                if add_ap is not None:
                    ga = gwp.tile([32, 512], F32, tag="ga")
                    gb = gwp.tile([32, 512], F32, tag="gb")
                    nc.vector.tensor_add(ga[:], h0, add_ap[:, 0:512])
                    nc.vector.tensor_add(gb[:], h1, add_ap[:, 512:GL])
                    h0, h1 = ga[:], gb[:]
                act = nc.scalar.activation
                if_t = gwp.tile([32, 2 * SL], F32, tag="ift")
                gt_t = gwp.tile([32, SL], F32, tag="gtt")
                o_t = gwp.tile([32, SL], F32, tag="ot")
                act(if_t[:], h0, SIG)                  # i, f fused
                act(gt_t[:], h1[:, 0:SL], TANH)
                act(o_t[:], h1[:, SL:2 * SL], SIG)
                tmp = gwp.tile([32, SL], F32, tag="tmp")
                nc.vector.tensor_mul(tmp[:], if_t[:, 0:SL], gt_t[:])
                c_new = cstp.tile([32, SL], F32, tag="c" + ltag)
                if c_old is not None:
                    cmul = gwp.tile([32, SL], F32, tag="cmul")
                    nc.gpsimd.tensor_mul(cmul[:], if_t[:, SL:2 * SL],
                                         c_old[:])
                    nc.vector.tensor_add(c_new[:], cmul[:], tmp[:])
                else:
                    nc.vector.tensor_copy(c_new[:], tmp[:])
                tanh_c = gwp.tile([32, SL], F32, tag="tanhc")
                act(tanh_c[:], c_new[:], TANH)
                hb = gwp.tile([32, SL], BF16, tag="hb")
                nc.vector.tensor_mul(hb[:], o_t[:], tanh_c[:])
                if out_idx is not None:
                    nc.sync.dma_start(chunks_out[out_idx], hb[:])
                # PE transpose to feature-partition layout; cheap on the
                # tensor queue (sits between this step's matmuls and the
                # next step's covered Whh half)
                pst = pstp.tile([128, 2 * NB], BF16, tag="pst")
                for k2 in range(2):
                    nc.tensor.matmul(
                        pst[:, k2 * NB:(k2 + 1) * NB],
                        hb[:, k2 * 128:(k2 + 1) * 128], ident_sb[:],
                        is_transpose=True, start=True, stop=True,
                        skip_group_check=True)
                ht = hslp.tile([128, 2 * NB], BF16, tag="hsl")
                nc.vector.tensor_copy(ht[:], pst[:])
                return c_new, ht

            def bias_mm(psums, bias_t, m):
                """Accumulate bias into psums via K=1 matmul (opens the
                accumulation group: start=True)."""
                for n in range(2):
                    nc.tensor.matmul(
                        psums[n][:], ones_sb[0:1, 0:m],
                        bias_t[0:1, n * 512:(n + 1) * 512],
                        start=True, stop=False, skip_group_check=True)

            def bulk_u(lhs_fn, rhs_fn, bias_t):
                """U[t] = X[t] @ Wih_l^T + b for 4 steps, one M=128 pass.
                uts[1..3] are base-0 [32, GL] fp32 tiles; uts[0] None
                (use u_sb[0:32])."""
                psums = [psup.tile([128, 512], F32, tag="psu",
                                   name=f"psu{n_}") for n_ in range(2)]
                bias_mm(psums, bias_t, 128)
                for kt in range(NT):
                    rhs = rhs_fn(kt)
                    lhs = lhs_fn(kt)
                    for n in range(2):
                        nc.tensor.matmul(
                            psums[n][:], lhs, rhs[:, n * 512:(n + 1) * 512],
                            start=False, stop=(kt == NT - 1),
                            skip_group_check=True)
                u_sb = usbp.tile([128, GL], F32, tag="usb")
                for n in range(2):
                    nc.vector.tensor_copy(u_sb[:, n * 512:(n + 1) * 512],
                                          psums[n][:])
                return u_sb

            def whh_matmuls(h_lhs_fn, whh_sb, extra=None, bias_t=None,
                            mid_hook=None):
                """K=2048 accumulation vs resident weights -> [psg0, psg1].
                extra: (lhs_fn, wih_sb) second K=2048 accumulation.
                bias_t: bias row accumulated first via K=1 matmul.
                mid_hook: emitted between the two accumulations (fills the
                tensor queue while extra's operand is still in flight)."""
                psums = [psgp.tile([32, 512], F32, tag="psg",
                                   name=f"psg{n_}") for n_ in range(2)]
                first = True
                if bias_t is not None:
                    bias_mm(psums, bias_t, 32)
                    first = False
                # n-major order: psg0 (i,f gates) stops before psg1, so
                # the cell's sigmoid + f*c start under the g,o matmuls
                for n in range(2):
                    for kt in range(NT):
                        nc.tensor.matmul(
                            psums[n][:], h_lhs_fn(kt),
                            whh_sb[:, kt * GL + n * 512:
                                   kt * GL + n * 512 + 512],
                            start=(first and kt == 0),
                            stop=(extra is None and kt == NT - 1),
                            skip_group_check=True)
                if mid_hook is not None:
                    mid_hook()
                if extra is not None:
                    lhs2, wih_sb = extra
                    for n in range(2):
                        for kt in range(NT):
                            nc.tensor.matmul(
                                psums[n][:], lhs2(kt),
                                wih_sb[:, kt * GL + n * 512:
                                       kt * GL + n * 512 + 512],
                                start=False, stop=(kt == NT - 1),
                                skip_group_check=True)
                return psums

            def wih_matmuls(lhs_fn, wih_sb, bias_t):
                """K=2048 accumulation vs resident Wih only (layer-1 t0)."""
                psums = [psgp.tile([32, 512], F32, tag="psg",
                                   name=f"psgw{n_}") for n_ in range(2)]
                bias_mm(psums, bias_t, 32)
                for n in range(2):
                    for kt in range(NT):
                        nc.tensor.matmul(
                            psums[n][:], lhs_fn(kt),
                            wih_sb[:, kt * GL + n * 512:
                                   kt * GL + n * 512 + 512],
                            start=False, stop=(kt == NT - 1),
                            skip_group_check=True)
                return psums

            def load_xin(x_dram, name):
                # host pre-laid-out [p, kt*128 + t*32 + b]; one linear DMA
                t_ = xinp.tile([128, NT * 128], BF16, tag="xin", name=name)
                nc.sync.dma_start(t_[:], x_dram[:, :])
                return t_

            def xin_block(x_sb, kt, t):
                return x_sb[:, kt * 128 + NB * t: kt * 128 + NB * t + NB]

            def dual_scan(l0_init_lhs, u_sb, whh0, whh1, wih1, b1,
                          c1_init, c2_init, x2t_out, h2_init_big,
                          zero_init, store_de):
                """Wavefront over both layers: layer-0 (bulk-U + Whh0) and
                layer-1 (step-wise: Wih1 @ h1_t + Whh1 @ h2_{t-1}).
                Layer-1's matmuls fill layer-0's AllGather windows."""
                c1p, c2p = c1_init, c2_init
                h2_prev = h2_init_big
                for t in range(4):
                    # ---- layer 0 step t ----
                    if zero_init and t == 0:
                        c1p, ht = cell(u_sb[0:32, :], None, None, "1")
                    else:
                        if t == 0:
                            lhs = l0_init_lhs
                        else:
                            lhs = lambda kt: x2t_block(x2t_out, kt, t - 1)
                        psums = whh_matmuls(lhs, whh0)
                        c1p, ht = cell(psums, u_sb[32 * t:32 * t + 32, :],
                                       c1p, "1")
                    cout = ag(ht)
                    x2t_from_ag(cout, x2t_out, t)
                    # ---- layer 1 step t ----
                    wih_lhs = lambda kt: x2t_block(x2t_out, kt, t)
                    if zero_init and t == 0:
                        psums = wih_matmuls(wih_lhs, wih1, b1)
                        c2p, ht = cell(psums, None, None, "2")
                    else:
                        h2b = h2_prev
                        psums = whh_matmuls(
                            lambda kt: big_block(h2b, kt), whh1,
                            extra=(wih_lhs, wih1), bias_t=b1)
                        c2p, ht = cell(
                            psums, None, c2p, "2",
                            out_idx=(0 if store_de and t == 3 else None))
                    cout = ag(ht)
                    h2_prev = big_from_ag(cout, h2qp, "h2",
                                          [nc.sync] * 4)
                return c1p, c2p, h2_prev

            # =========================================================
            # Phase E: encoder (batch 32 = [x2 fwd chain, x1-rev chain])
            # =========================================================
            xe_sb = load_xin(xET, "xe_sb")          # first on sync ring
            eb0 = load_bias(eB, 0, "eb0")
            eb1 = load_bias(eB, 1, "eb1")
            db0 = load_bias(dB, 0, "db0")
            db1 = load_bias(dB, 1, "db1")
            whh_e0 = load_w(eWhh, 0, "whh_e0", nc.sync)
            whh_e1 = load_w(eWhh, 1, "whh_e1", nc.sync)

            def wstream(w_dram, l, kt, name):
                wt = wstrp.tile([128, GL], BF16, tag="wstr", name=name)
                nc.scalar.dma_start(
                    wt[:], w_dram[l, kt * 128:(kt + 1) * 128, :])
                return wt[:]

            u_e = bulk_u(
                lambda kt: xe_sb[:, kt * 128:(kt + 1) * 128],
                lambda kt: wstream(eWih, 0, kt, f"we{kt}"), eb0)
            # D1 bulk hoisted here: fills the cold-collective window
            # before the first encoder AllGather completes
            xd_sb = load_xin(xDT, "xd_sb")
            u_d = bulk_u(
                lambda kt: xd_sb[:, kt * 128:(kt + 1) * 128],
                lambda kt: wstream(dWih, 0, kt, f"wd{kt}"), db0)
            wih_e1 = load_w(eWih, 1, "wih_e1", nc.scalar)
            whh_d0 = load_w(dWhh, 0, "whh_d0", nc.scalar)
            cb_sb = biasp.tile([64, C], F32, tag="cb", name="cb_sb", bufs=1)
            nc.gpsimd.dma_start(cb_sb[:], cB[:])
            cwt_sb = cwtp.tile([128, 8 * C], BF16, tag="cwt")
            nc.gpsimd.dma_start(
                cwt_sb[:].rearrange("p (j o) -> p j o", j=8),
                cWT.rearrange("(j p) o -> p j o", p=128))
            x2t_e = (x2tp.tile([128, 4 * 256], BF16, tag="x2ta",
                               name="x2t_ea"),
                     x2tp.tile([128, 4 * 256], BF16, tag="x2tb",
                               name="x2t_eb"))
            c_e1, c_e2, h2_big = dual_scan(
                None, u_e, whh_e0, whh_e1, wih_e1, eb1,
                None, None, x2t_e, None, True, False)

            # =========================================================
            # Phase D1: decoder consume (batch = [x1 fwd, x2-rev])
            # =========================================================
            wih_d0 = load_w(dWih, 0, "wih_d0", nc.scalar)
            whh_d1 = load_w(dWhh, 1, "whh_d1", nc.scalar)
            wih_d1 = load_w(dWih, 1, "wih_d1", nc.scalar)
            x2t_d = (x2tp.tile([128, 4 * 256], BF16, tag="x2ta",
                               name="x2t_da"),
                     x2tp.tile([128, 4 * 256], BF16, tag="x2tb",
                               name="x2t_db"))
            c1, c2, h2_big = dual_scan(
                lambda kt: x2t_block(x2t_e, kt, 3), u_d,
                whh_d0, whh_d1, wih_d1, db1,
                c_e1, c_e2, x2t_d, h2_big, False, True)

            # =========================================================
            # Phase D2: autoregressive decoder (7 steps, zero streaming)
            # =========================================================
            conv_tiles = {"de": h2_big}
            h1_big = None
            ENGS4 = [nc.sync, nc.scalar, nc.sync, nc.scalar]

            def conv_hook(w):
                """One conv output column: 8 gathered lhs tiles (gpsimd
                copies) + 8 matmuls, accumulated into pcv, then bias +
                leaky-relu on gpsimd.  Issued inside the AR loop's tensor
                idle window."""
                b1 = [conv_tiles["de"], conv_tiles["ar0"],
                      conv_tiles["ar1"], conv_tiles["ar2"]]
                b2 = [conv_tiles["ar2"], conv_tiles["ar1"],
                      conv_tiles["ar0"], conv_tiles["de"]]

                def hook():
                    pcv = psup.tile([128, 512], F32, tag="psu",
                                    name=f"pcv{w}")
                    first = True
                    for br, src in ((0, b1[w]), (1, b2[w])):
                        for j in range(4):
                            st = gwp.tile([128, 64], BF16, tag="cvl",
                                          bufs=2, name=f"cvl{w}_{br}_{j}")
                            for h in range(4):
                                # global kt = j + 4h lives in quarter h
                                nc.gpsimd.tensor_copy(
                                    st[:, h * 16:h * 16 + 16],
                                    src[h][:, j * NB + 16 * br:
                                           j * NB + 16 * br + 16])
                            nc.tensor.matmul(
                                pcv[0:64, :], st[:],
                                cwt_sb[:, (4 * br + j) * C:
                                       (4 * br + j + 1) * C],
                                start=first, stop=(br == 1 and j == 3),
                                skip_group_check=True)
                            first = False
                    cvs = gwp.tile([64, C], F32, tag="g", name=f"cvs{w}")
                    nc.vector.tensor_add(cvs[:], pcv[0:64, :], cb_sb[:])
                    cvo = gwp.tile([64, C], F32, tag="g2", name=f"cvo{w}")
                    nc.vector.tensor_scalar_mul(cvo[:], cvs[:], 0.2)
                    nc.vector.tensor_max(cvo[:], cvo[:], cvs[:])
                    nc.gpsimd.dma_start(convout[w], cvo[:])
                return hook

            for t in range(NAR):
                h2b, h1b = h2_big, h1_big
                if t == 0:
                    l0_lhs = lambda kt: x2t_block(x2t_d, kt, 3)
                else:
                    l0_lhs = lambda kt: big_block(h1b, kt)
                psums = whh_matmuls(
                    l0_lhs, whh_d0,
                    extra=(lambda kt: big_block(h2b, kt), wih_d0),
                    bias_t=db0)
                c1, ht = cell(psums, None, c1, "1")
                cout = ag(ht)
                h1_big = big_from_ag(cout, h1qp, "h1", ENGS4)

                h1b2 = h1_big
                psums = whh_matmuls(
                    lambda kt: big_block(h2b, kt), whh_d1,
                    extra=(lambda kt: big_block(h1b2, kt), wih_d1),
                    bias_t=db1,
                    mid_hook=(conv_hook(t - 3) if t >= 3 else None))
                c2, ht = cell(psums, None, c2, "2", out_idx=t + 1)
                if t < NAR - 1:
                    cout = ag(ht)
                    h2_big = big_from_ag(cout, h2qp, "h2", ENGS4)
                    if t < 3:
                        conv_tiles[f"ar{t}"] = h2_big

    nc.compile()
    return nc


def _prep_inputs(x1, x2, enc_Wih, enc_Whh, enc_bih, enc_bhh,
                 dec_Wih, dec_Whh, dec_bih, dec_bhh, conv_W, conv_b):
    def colvecs(x):
        return [np.ascontiguousarray(x[:, :, :, t].reshape(B, L))
                for t in range(4)]

    x1c, x2c = colvecs(x1), colvecs(x2)

    def ximg(xa):
        # [4, L, NB] -> SBUF image [128, kt*128 + t*32 + b]
        return np.ascontiguousarray(
            xa.reshape(4, NT, 128, NB).transpose(2, 1, 0, 3)
            .reshape(128, NT * 4 * NB)).astype(NPBF)

    xET = ximg(np.stack([
        np.concatenate([x2c[t], x1c[3 - t]], axis=0)[:, PERM].T
        for t in range(4)]))
    xDT = ximg(np.stack([
        np.concatenate([x1c[t], x2c[3 - t]], axis=0)[:, PERM].T
        for t in range(4)]))

    def prep_core(k, Wih, Whh, bih, bhh):
        rows = np.concatenate([g * L + PERM[k * SL:(k + 1) * SL]
                               for g in range(4)])
        wihT = np.stack([np.ascontiguousarray(Wih[l][rows][:, PERM].T)
                         for l in range(2)])
        whhT = np.stack([np.ascontiguousarray(Whh[l][rows][:, PERM].T)
                         for l in range(2)])
        bb = np.stack([(bih[l] + bhh[l])[rows] for l in range(2)])
        return (wihT.astype(NPBF), whhT.astype(NPBF),
                bb.reshape(2, 1, GL).astype(NPBF))

    cWT = np.ascontiguousarray(conv_W.T).astype(NPBF)
    cBr = np.broadcast_to(conv_b[None, :], (64, C)).copy().astype(np.float32)
    ident = np.eye(32, dtype=NPBF)

    in_maps = []
    for k in range(NCORES):
        eWihT, eWhhT, eBr = prep_core(k, enc_Wih, enc_Whh, enc_bih, enc_bhh)
        dWihT, dWhhT, dBr = prep_core(k, dec_Wih, dec_Whh, dec_bih, dec_bhh)
        in_maps.append({
            "xET": xET, "xDT": xDT,
            "eWih": eWihT, "eWhh": eWhhT, "eB": eBr,
            "dWih": dWihT, "dWhh": dWhhT, "dB": dBr,
            "cWT": cWT, "cB": cBr, "ident": ident,
        })
    return in_maps


def _postprocess(results, x1, x2):
    # gather chunk slices across cores: core k owns features [256k:256k+256)
    chunks = np.zeros((8, B * 2, L), np.float32)
    for k in range(NCORES):
        chunks[:, :, k * SL:(k + 1) * SL] = \
            results[k]["chunks_out"].astype(np.float32)
    convout = results[0]["convout"]

    def tochunk(t, half):
        v = chunks[t, half * B:(half + 1) * B, :]   # [16, L] dev order
        return v[:, IPERM].reshape(B, C, H)

    de1 = tochunk(0, 0)
    p1 = [tochunk(1 + j, 0) for j in range(NAR)]
    de2 = tochunk(0, 1)
    p2 = [tochunk(1 + j, 1) for j in range(NAR)]

    mid1 = np.stack([de1, p1[0], p1[1], p1[2]], axis=-1)
    tail1 = np.stack([p1[3], p1[4], p1[5], p1[6]], axis=-1)
    head2 = np.stack([p2[6], p2[5], p2[4], p2[3]], axis=-1)
    mid2 = np.stack([p2[2], p2[1], p2[0], de2], axis=-1)

    out = convout.reshape(4, 4, B, C).transpose(2, 3, 1, 0)
    out = np.ascontiguousarray(out, dtype=np.float32)
    return (out, np.asarray(x1), mid1, tail1, head2, mid2, np.asarray(x2))


def _run(in_maps, trace=False):
    if "nc" not in _CACHE:
        _CACHE["nc"] = _build_nc()
        _CACHE["tmpdir"] = tempfile.mkdtemp(prefix="lstmk_")
    nc = _CACHE["nc"]
    res = bass_utils.run_bass_kernel_spmd(
        nc, in_maps, core_ids=list(range(NCORES)), trace=trace,
        tmpdir=_CACHE["tmpdir"] if trace else None)
    return res


def kernel(**inputs):
    inputs = {k: np.asarray(v, dtype=np.float32) for k, v in inputs.items()}
    in_maps = _prep_inputs(**inputs)
    res = _run(in_maps, trace=False)
    return _postprocess(res.results, inputs["x1"], inputs["x2"])


def kernel_traced(**inputs):
    inputs = {k: np.asarray(v, dtype=np.float32) for k, v in inputs.items()}
    in_maps = _prep_inputs(**inputs)
    res = _run(in_maps, trace=True)
    return _postprocess(res.results, inputs["x1"], inputs["x2"]), res
